# revision 1
# baseline (speedup 1.0000x reference)
"""Causal single-head attention (B=4, S=2048, D=1024) on 8 Trainium2 cores.

Sharding: 2 cores per batch. Core parity p in {0,1} owns global query tiles
gq = 2t+p (t = 0..7), i.e. interleaved 128-query tiles. This makes the device
program identical on all 8 cores (only input data differs):
  - every core computes K^T and V projections for all 2048 keys of its batch
  - core p's t-th query tile attends (2t+2)*128 keys, with a parity-dependent
    additive mask input covering the last 256 key columns (causal diagonal)
Per-core phases:
  A1: K^T = W_K^T @ X^T       -> SBUF resident [128, 8, 2048] fp32r
  A2: V   = X @ W_V           -> SBUF resident [128, 16, 1024] fp32r
  A3: Q^T = (W_Q^T @ X_q^T)/32 -> DRAM scratch (SBUF too small in phase A)
  B:  flash attention per query tile: S = Q^T.T K^T (PSUM), +mask, exp (ACT,
      rowsum via accum), PE-transpose P, O += P^T.T V (PSUM), O *= 1/rowsum.
All matmuls run as fp32r (tf32-like, 1 cycle/row at N>=256), fp32 accumulate.
"""

import numpy as np

B, S, D = 4, 2048, 1024
NCORES = 8
DC = D // 128        # 8 contraction chunks
NKT = S // 128       # 16 key tiles
NQT = 8              # query tiles per core
SCALE = 1.0 / np.sqrt(np.float32(D))

_CACHE = {}


def _build(cfg=None):
    from contextlib import ExitStack

    from concourse import bacc
    import concourse.mybir as mybir
    import concourse.tile as tile

    cfg = cfg or {}
    WPOOL = cfg.get("wpool", 10)
    A2_COUTER = cfg.get("a2_couter", False)
    A3_COUTER = cfg.get("a3_couter", True)
    COPY_ALT = cfg.get("copy_alt", False)
    SWDGE_RT = cfg.get("swdge", False)
    V_COUTER = cfg.get("v_couter", False)

    FP32 = mybir.dt.float32
    FP32R = mybir.dt.float32r
    EXP = mybir.ActivationFunctionType.Exp
    COPY = mybir.ActivationFunctionType.Copy
    AX = mybir.AxisListType.X
    ADD = mybir.AluOpType.add

    nc = bacc.Bacc("TRN2", debug=False, num_devices=NCORES, dynamic_dma_scratch_size=4096)
    # X^T chunked layouts (host-prepared):
    #   xt   [kc, p, c, k]: 256-key chunks, all c contiguous per partition row
    #   xt512[kc, c, p, k]: 512-key chunks, per-c planes
    xt = nc.dram_tensor("xt", [8, 128, DC, 256], FP32R, kind="ExternalInput").ap()
    xt512 = nc.dram_tensor("xt512", [4, DC, 128, 512], FP32R, kind="ExternalInput").ap()
    xtq = nc.dram_tensor("xtq", [4, 128, DC, 256], FP32R, kind="ExternalInput").ap()
    wq = nc.dram_tensor("wq", [DC, 128, D], FP32R, kind="ExternalInput").ap()
    wk = nc.dram_tensor("wk", [DC, 128, D], FP32R, kind="ExternalInput").ap()
    wv = nc.dram_tensor("wv", [DC, 128, D], FP32R, kind="ExternalInput").ap()
    mask = nc.dram_tensor("mask", [128, 256], FP32, kind="ExternalInput").ap()
    ident_in = nc.dram_tensor("ident", [128, 128], FP32R, kind="ExternalInput").ap()
    o = nc.dram_tensor("o", [NQT, 128, D], FP32, kind="ExternalOutput").ap()

    with tile.TileContext(nc) as tc, ExitStack() as ctx:
        const = ctx.enter_context(tc.tile_pool(name="const", bufs=1))
        resident = ctx.enter_context(tc.tile_pool(name="resident", bufs=1))
        dram = ctx.enter_context(tc.tile_pool(name="dram", bufs=1, space="DRAM"))

        ident = const.tile([128, 128], FP32R)
        mask_sb = const.tile([128, 256], FP32)

        kt_sb = resident.tile([128, DC, S], FP32R)       # K^T [d | dc, keys]
        v_sb = resident.tile([128, NKT, D], FP32R)       # V   [k | ktile, dv]
        qt_dram = dram.tile([4, 128, DC, 256], FP32R)

        # ---------------- Phase A: projections ----------------
        with tc.tile_pool(name="wpool", bufs=WPOOL) as wp, \
             tc.tile_pool(name="xchS", bufs=cfg.get("xsbufs", 2)) as xsp, \
             tc.tile_pool(name="apsum", bufs=8, space="PSUM") as aps:

            def psum_copy(dst, src, i):
                if COPY_ALT and i % 2 == 1:
                    nc.scalar.copy(dst, src)
                else:
                    nc.vector.tensor_copy(dst, src)

            def kproj_256_couter(xch, kc256):
                """c-outer: wk[c] last-used at step c -> early release for wv loads."""
                kpss = [aps.tile([128, 512], FP32, tag="ps", name=f"kpo{m}") for m in range(DC)]
                for c in range(DC):
                    for m in range(DC):
                        nc.tensor.matmul(
                            kpss[m][:, 0:256],
                            wk_t[c][:, m * 128 : (m + 1) * 128],
                            xch[:, c, :],
                            start=(c == 0),
                            stop=(c == DC - 1),
                            skip_group_check=True,
                        )
                for m in range(DC):
                    psum_copy(
                        kt_sb[:, m, kc256 * 256 : (kc256 + 1) * 256], kpss[m][:, 0:256], m
                    )

            def kproj_256(xch, kc256):
                """K^T for one 256-key chunk held in [128, DC, 256] tile."""
                for m in range(DC):
                    kps = aps.tile([128, 512], FP32, tag="ps", name="kps")
                    for c in range(DC):
                        nc.tensor.matmul(
                            kps[:, 0:256],
                            wk_t[c][:, m * 128 : (m + 1) * 128],
                            xch[:, c, :],
                            start=(c == 0),
                            stop=(c == DC - 1),
                        )
                    psum_copy(
                        kt_sb[:, m, kc256 * 256 : (kc256 + 1) * 256], kps[:, 0:256], m
                    )

            def vproj_256(xch, kc256, couter=False):
                """V for one 256-key chunk held in [128, DC, 256] tile."""
                if couter:
                    vpss = [
                        aps.tile([128, 512], FP32, tag="ps", name=f"vps{j}{h}")
                        for j in range(2)
                        for h in range(2)
                    ]
                    for c in range(DC):
                        for jh in range(4):
                            nc.tensor.matmul(
                                vpss[jh][:],
                                xch[:, c, (jh // 2) * 128 : (jh // 2 + 1) * 128],
                                wv_t[c][:, (jh % 2) * 512 : (jh % 2 + 1) * 512],
                                start=(c == 0),
                                stop=(c == DC - 1),
                                skip_group_check=True,
                            )
                    for jh in range(4):
                        psum_copy(
                            v_sb[:, kc256 * 2 + jh // 2, (jh % 2) * 512 : (jh % 2 + 1) * 512],
                            vpss[jh][:],
                            jh,
                        )
                    return
                for j in range(2):
                    for h in range(2):
                        vps = aps.tile([128, 512], FP32, tag="ps", name="vps")
                        for c in range(DC):
                            nc.tensor.matmul(
                                vps[:],
                                xch[:, c, j * 128 : (j + 1) * 128],
                                wv_t[c][:, h * 512 : (h + 1) * 512],
                                start=(c == 0),
                                stop=(c == DC - 1),
                            )
                        psum_copy(
                            v_sb[:, kc256 * 2 + j, h * 512 : (h + 1) * 512], vps[:], j * 2 + h
                        )

            # A1: K^T[m, k] = sum_c W_K[c, m].T @ X^T[c, k]
            wk_t = []
            WK_SWDGE = cfg.get("wk_swdge", 1)
            for c in range(DC):
                w_tile = wp.tile([128, D], FP32R, name=f"wk{c}", tag="w")
                eng = nc.gpsimd if c < WK_SWDGE else nc.scalar
                eng.dma_start(w_tile[:], wk[c])
                wk_t.append(w_tile)
            with tc.tile_pool(name="xch1", bufs=2) as xp1:
                for kc in range(3):  # first three 512-key chunks
                    xch = xp1.tile([128, DC, 512], FP32R, tag="x1", name="xch1")
                    for c in range(DC):
                        nc.sync.dma_start(xch[:, c, :], xt512[kc, c])
                    if kc == 0:
                        # c-outer: first matmul needs only wk[0] + one X slice
                        kpss = [aps.tile([128, 512], FP32, tag="ps", name=f"kps{m}") for m in range(DC)]
                        for c in range(DC):
                            for m in range(DC):
                                nc.tensor.matmul(
                                    kpss[m][:],
                                    wk_t[c][:, m * 128 : (m + 1) * 128],
                                    xch[:, c, :],
                                    start=(c == 0),
                                    stop=(c == DC - 1),
                                    skip_group_check=True,
                                )
                        for m in range(DC):
                            psum_copy(
                                kt_sb[:, m, kc * 512 : (kc + 1) * 512], kpss[m][:], m
                            )
                    else:
                        for m in range(DC):
                            kps = aps.tile([128, 512], FP32, tag="ps")
                            for c in range(DC):
                                nc.tensor.matmul(
                                    kps[:],
                                    wk_t[c][:, m * 128 : (m + 1) * 128],
                                    xch[:, c, :],
                                    start=(c == 0),
                                    stop=(c == DC - 1),
                                )
                            psum_copy(
                                kt_sb[:, m, kc * 512 : (kc + 1) * 512], kps[:], m
                            )
                # last 512 keys as two 256-key chunks from the shared pool so
                # A2 can reuse the tiles without any phase-boundary DMA
                xt6_t = xsp.tile([128, DC, 256], FP32R, tag="x", name="xt6")
                nc.sync.dma_start(xt6_t[:], xt[6])
                kproj_256(xt6_t, 6)
                xt7_t = xsp.tile([128, DC, 256], FP32R, tag="x", name="xt7")
                nc.sync.dma_start(xt7_t[:], xt[7])
                if cfg.get("a1_tail_couter", True):
                    kproj_256_couter(xt7_t, 7)
                else:
                    kproj_256(xt7_t, 7)

            # A2: V[k, n] = sum_c X^T[c, k].T @ W_V[c, n]; reverse key order,
            # first two chunks reuse A1's resident tiles
            wv_t = []
            for c in range(DC):
                w_tile = wp.tile([128, D], FP32R, name=f"wv{c}", tag="w")
                nc.scalar.dma_start(w_tile[:], wv[c])
                wv_t.append(w_tile)
            stage_cm = tc.tile_pool(name="stage", bufs=cfg.get("stbufs", 3))
            stp = stage_cm.__enter__()
            xq3_t = None
            vproj_256(xt7_t, 7, couter=V_COUTER)
            vproj_256(xt6_t, 6)
            for kc in (5, 4, 3, 2, 1, 0):
                xch = xsp.tile([128, DC, 256], FP32R, tag="x", name="xch2")
                nc.sync.dma_start(xch[:], xt[kc])
                if kc == 5:
                    # prefetch A3's first query chunk while A2 still runs
                    xq3_t = stp.tile([128, DC, 256], FP32R, tag="qs", name="xq3")
                    nc.sync.dma_start(xq3_t[:], xtq[3])
                vproj_256(xch, kc, couter=(kc == 0 and cfg.get("a2_tail_couter", True)))

            # A3: Q^T[m, q] = (sum_c W_Q[c, m].T @ Xq^T[c, q]) * SCALE -> DRAM
            wq_t = []
            for c in range(DC):
                w_tile = wp.tile([128, D], FP32R, name=f"wq{c}", tag="w")
                nc.scalar.dma_start(w_tile[:], wq[c])
                wq_t.append(w_tile)
            if True:
                for qc in (3, 2, 1, 0):
                    if qc == 3:
                        xch = xq3_t
                    else:
                        xch = xsp.tile([128, DC, 256], FP32R, tag="x", name="xch3")
                        nc.sync.dma_start(xch[:], xtq[qc])
                    qstage = stp.tile([128, DC, 256], FP32R, tag="qs")
                    if qc == 3 and A3_COUTER:
                        qpss = [aps.tile([128, 512], FP32, tag="ps", name=f"qps{m}") for m in range(DC)]
                        for c in range(DC):
                            for m in range(DC):
                                nc.tensor.matmul(
                                    qpss[m][:, 0:256],
                                    wq_t[c][:, m * 128 : (m + 1) * 128],
                                    xch[:, c, :],
                                    start=(c == 0),
                                    stop=(c == DC - 1),
                                    skip_group_check=True,
                                )
                        for m in range(DC):
                            if cfg.get("q3_alt", False) and m % 2 == 1:
                                nc.vector.tensor_scalar_mul(
                                    qstage[:, m, :], qpss[m][:, 0:256], float(SCALE)
                                )
                            else:
                                nc.scalar.activation(qstage[:, m, :], qpss[m][:, 0:256], COPY, scale=float(SCALE))
                    else:
                        for m in range(DC):
                            qps = aps.tile([128, 512], FP32, tag="ps")
                            for c in range(DC):
                                nc.tensor.matmul(
                                    qps[:, 0:256],
                                    wq_t[c][:, m * 128 : (m + 1) * 128],
                                    xch[:, c, :],
                                    start=(c == 0),
                                    stop=(c == DC - 1),
                                )
                            nc.scalar.activation(qstage[:, m, :], qps[:, 0:256], COPY, scale=float(SCALE))
                    nc.scalar.dma_start(qt_dram[qc], qstage[:])
            stage_cm.__exit__(None, None, None)

        # ---------------- Phase B: attention ----------------
        with tc.tile_pool(name="qpool", bufs=1) as qp, \
             tc.tile_pool(name="ppool", bufs=cfg.get("pbufs", 2)) as pp, \
             tc.tile_pool(name="ptpool", bufs=cfg.get("ptbufs", 2)) as ptp, \
             tc.tile_pool(name="small", bufs=4) as smp, \
             tc.tile_pool(name="obuf", bufs=cfg.get("obbufs", 2)) as obp, \
             tc.tile_pool(name="spsum", bufs=cfg.get("sbufs", 3), space="PSUM") as sps, \
             tc.tile_pool(name="tpsum", bufs=cfg.get("tbufs", 1), space="PSUM") as tps, \
             tc.tile_pool(name="opsum", bufs=cfg.get("obufs", 2), space="PSUM") as ops:

            nc.sync.dma_start(ident[:], ident_in[:])
            nc.sync.dma_start(mask_sb[:], mask[:])
            qt_sb = qp.tile([128, DC, 4, 256], FP32R)
            if cfg.get("split_qt3", False):
                # split the critical first reload so t=7's S can start sooner
                nc.scalar.dma_start(qt_sb[:, 0:4, 3, :], qt_dram[3][:, 0:4, :])
                nc.scalar.dma_start(qt_sb[:, 4:8, 3, :], qt_dram[3][:, 4:8, :])
                for qc in (2, 1, 0):
                    nc.scalar.dma_start(qt_sb[:, :, qc, :], qt_dram[qc])
            else:
                for qc in (3, 2, 1, 0):
                    nc.scalar.dma_start(qt_sb[:, :, qc, :], qt_dram[qc])

            t_order = cfg.get("t_order", list(reversed(range(NQT))))
            for t in t_order:
                nk = (2 * t + 2) * 128
                nf, rem = divmod(nk, 512)
                widths = [512] * nf + ([rem] if rem else [])
                nch = len(widths)

                o_ps = ops.tile([128, D], FP32, tag="o")
                rsparts = smp.tile([128, 4], FP32, tag="rs")

                for ci, w in enumerate(widths):
                    kbase = ci * 512
                    last = ci == nch - 1
                    s_ps = sps.tile([128, 512], FP32, tag="s")
                    for c in range(DC):
                        nc.tensor.matmul(
                            s_ps[:, 0:w],
                            qt_sb[:, c, t // 2, (t % 2) * 128 : (t % 2) * 128 + 128],
                            kt_sb[:, c, kbase : kbase + w],
                            start=(c == 0),
                            stop=(c == DC - 1),
                        )
                    if last:
                        nc.vector.tensor_tensor(
                            s_ps[:, w - 256 : w], s_ps[:, w - 256 : w], mask_sb[:], ADD
                        )
                    p_sb = pp.tile([128, 512], FP32R, tag="p")
                    nc.scalar.activation(
                        p_sb[:, 0:w], s_ps[:, 0:w], EXP,
                        accum_out=rsparts[:, ci : ci + 1],
                    )
                    pt_ps = tps.tile([128, 512], FP32R, tag="pt")
                    for j in range(w // 128):
                        nc.tensor.transpose(
                            pt_ps[:, j * 128 : (j + 1) * 128],
                            p_sb[:, j * 128 : (j + 1) * 128],
                            ident[:],
                        )
                    pt_sb = ptp.tile([128, 512], FP32R, tag="pts")
                    nc.vector.tensor_copy(pt_sb[:, 0:w], pt_ps[:, 0:w])
                    for j in range(w // 128):
                        kt_idx = kbase // 128 + j
                        for h in range(2):
                            nc.tensor.matmul(
                                o_ps[:, h * 512 : (h + 1) * 512],
                                pt_sb[:, j * 128 : (j + 1) * 128],
                                v_sb[:, kt_idx, h * 512 : (h + 1) * 512],
                                start=(ci == 0 and j == 0),
                                stop=(last and j == w // 128 - 1),
                                skip_group_check=True,
                            )

                rs_tot = smp.tile([128, 1], FP32, tag="rst")
                nc.vector.tensor_reduce(rs_tot[:], rsparts[:, 0:nch], axis=AX, op=ADD)
                recip = smp.tile([128, 1], FP32, tag="rcp")
                nc.vector.reciprocal(recip[:], rs_tot[:])
                o_sb = obp.tile([128, D], FP32, tag="ob")
                for h in range(2):
                    nc.scalar.activation(
                        o_sb[:, h * 512 : (h + 1) * 512],
                        o_ps[:, h * 512 : (h + 1) * 512],
                        COPY,
                        scale=recip[:],
                    )
                    nc.sync.dma_start(o[t][:, h * 512 : (h + 1) * 512], o_sb[:, h * 512 : (h + 1) * 512])

    nc.compile()
    return nc


def _make_runner(nc):
    """Cached jitted 8-core runner (no donation; avoids per-call re-jit)."""
    import jax
    import numpy as np_
    from jax.sharding import Mesh, PartitionSpec
    from jax.experimental.shard_map import shard_map

    from concourse import mybir
    from concourse.bass2jax import (
        _bass_exec_p,
        install_neuronx_cc_hook,
        partition_id_tensor,
    )

    install_neuronx_cc_hook()
    partition_name = nc.partition_id_tensor.name if nc.partition_id_tensor else None
    in_names, out_names, out_avals = [], [], []
    for alloc in nc.m.functions[0].allocations:
        if not isinstance(alloc, mybir.MemoryLocationSet):
            continue
        name = alloc.memorylocations[0].name
        if alloc.kind == "ExternalInput":
            if name != partition_name:
                in_names.append(name)
        elif alloc.kind == "ExternalOutput":
            out_names.append(name)
            out_avals.append(
                jax.core.ShapedArray(
                    tuple(alloc.tensor_shape), mybir.dt.np(alloc.dtype)
                )
            )
    n_params = len(in_names)
    all_in = list(in_names) + list(out_names)
    if partition_name is not None:
        all_in.append(partition_name)

    def _body(*args):
        operands = list(args)
        if partition_name is not None:
            operands.append(partition_id_tensor())
        return tuple(
            _bass_exec_p.bind(
                *operands,
                out_avals=tuple(out_avals),
                in_names=tuple(all_in),
                out_names=tuple(out_names),
                lowering_input_output_aliases=(),
                sim_require_finite=True,
                sim_require_nnan=True,
                nc=nc,
            )
        )

    devices = jax.devices()[:NCORES]
    mesh = Mesh(np_.asarray(devices), ("core",))
    spec = PartitionSpec("core")
    fn = jax.jit(
        shard_map(
            _body,
            mesh=mesh,
            in_specs=(spec,) * (n_params + len(out_names)),
            out_specs=(spec,) * len(out_names),
            check_rep=False,
        ),
        keep_unused=True,
    )

    def run(in_maps):
        concat_in = [
            np_.concatenate([np_.asarray(m[nm]) for m in in_maps], axis=0)
            for nm in in_names
        ]
        zeros = [
            np_.zeros((NCORES * a.shape[0], *a.shape[1:]), a.dtype) for a in out_avals
        ]
        outs = fn(*concat_in, *zeros)
        return [
            {
                nm: np_.asarray(outs[i]).reshape(NCORES, *out_avals[i].shape)[c]
                for i, nm in enumerate(out_names)
            }
            for c in range(NCORES)
        ]

    return run


def kernel(embeddings, W_Q, W_K, W_V):
    from concourse.bass_utils import run_bass_kernel_spmd

    emb = np.ascontiguousarray(np.asarray(embeddings, dtype=np.float32))
    wq_np = np.ascontiguousarray(np.asarray(W_Q, dtype=np.float32)).reshape(DC, 128, D)
    wk_np = np.ascontiguousarray(np.asarray(W_K, dtype=np.float32)).reshape(DC, 128, D)
    wv_np = np.ascontiguousarray(np.asarray(W_V, dtype=np.float32)).reshape(DC, 128, D)

    if "nc" not in _CACHE:
        _CACHE["nc"] = _build()
    nc = _CACHE["nc"]

    tri = (np.arange(128)[:, None] >= np.arange(128)[None, :]).astype(np.float32)
    neg = np.float32(-1e9)
    masks = []
    for p in range(2):
        m = np.zeros((128, 256), dtype=np.float32)
        if p == 0:
            m[:, 0:128] = np.where(tri > 0, 0.0, neg)
            m[:, 128:256] = neg
        else:
            m[:, 0:128] = 0.0
            m[:, 128:256] = np.where(tri > 0, 0.0, neg)
        masks.append(m)
    ident_np = np.eye(128, dtype=np.float32)

    # per-batch X^T layouts (shared by the two cores of each batch)
    xt_b, xt512_b, xtb = [], [], []
    for b in range(B):
        x_t = np.ascontiguousarray(emb[b].T)  # [D, S]
        xtb.append(x_t)
        xt_b.append(
            np.ascontiguousarray(x_t.reshape(DC, 128, 8, 256).transpose(2, 1, 0, 3))
        )
        xt512_b.append(
            np.ascontiguousarray(x_t.reshape(DC, 128, 4, 512).transpose(2, 0, 1, 3))
        )

    in_maps = []
    for core in range(NCORES):
        b, p = divmod(core, 2)
        x_t = xtb[b]
        xt_np = xt_b[b]
        xt512_np = xt512_b[b]
        q_tiles = [x_t[:, (2 * t + p) * 128 : (2 * t + p + 1) * 128] for t in range(NQT)]
        xtq_np = np.concatenate(q_tiles, axis=1)  # [D, 1024]
        xtq_c = np.ascontiguousarray(
            xtq_np.reshape(DC, 128, 4, 256).transpose(2, 1, 0, 3)
        )
        in_maps.append(
            {
                "xt": xt_np,
                "xt512": xt512_np,
                "xtq": xtq_c,
                "wq": wq_np,
                "wk": wk_np,
                "wv": wv_np,
                "mask": masks[p],
                "ident": ident_np,
            }
        )

    global _last_in_maps
    _last_in_maps = in_maps
    results = None
    try:
        if "runner" not in _CACHE:
            _CACHE["runner"] = _make_runner(nc)
        results = _CACHE["runner"](in_maps)
    except Exception:
        _CACHE.pop("runner", None)
    if results is None:
        import time as _time

        for attempt in range(2):
            try:
                results = run_bass_kernel_spmd(
                    nc, in_maps, core_ids=list(range(NCORES))
                ).results
                break
            except Exception:
                if attempt == 1:
                    raise
                _time.sleep(3.0)

    out = np.empty((B, S, D), dtype=np.float32)
    for core in range(NCORES):
        b, p = divmod(core, 2)
        o_core = results[core]["o"]  # [NQT, 128, D]
        for t in range(NQT):
            gq = 2 * t + p
            out[b, gq * 128 : (gq + 1) * 128, :] = o_core[t]
    return out



# revision 2
# speedup vs baseline: 1.0834x; 1.0834x over previous
"""Causal single-head attention (B=4, S=2048, D=1024) on 8 Trainium2 cores.

Sharding: 2 cores per batch, interleaved KEY tiles (core parity p owns global
key tiles 2j+p). Each core computes K^T,V for its 1024 keys, Q^T for all 2048
queries, then S^T-major flash attention over its keys, producing UNNORMALIZED
partial outputs + partial rowsums. The host merges the two cores of a batch:
out = (Ohat_e + Ohat_o) / (rs_e + rs_o).  (No max-subtraction, like the
fp32r baseline; exp arguments are small enough.)

Numerics / speed:
  - Projections run as fp8(e4m3) DoubleRow matmuls with error compensation:
    X^T and 64*W are split on the HOST into hi + lo e4m3 parts and the three
    products hh, hl, lh are accumulated in fp32 PSUM (the tiny lo*lo term is
    dropped). DoubleRow contracts 256 elements per 0.5 cyc/row -> 3x fp32r
    throughput for ~1e-3 relative error. PSUM->SBUF copies rescale by 1/64.
  - Attention (S^T and PV) runs in bf16 (1 cyc/row, same as fp32r, but
    transpose-free): S^T = K^T.T Q^T is computed key-major so exp() writes
    P^T directly in the layout PV needs; rowsums come from an extra ap=1
    matmul against a ones vector reusing the PV stationary.
  - The program is SPMD-identical on all 8 cores; parity enters only through
    the host-packed column permutation of X^T and two mask tiles (diagonal
    tri mask + a phantom-tile mask that zeroes the odd core's extra tile).
"""

import numpy as np

B, S, D = 4, 2048, 1024
NCORES = 8
NT = 16             # 128-row tiles per sequence
SCALE = 1.0 / np.sqrt(np.float32(D))
WSC = 64.0          # weight pre-scale for fp8 (W ~ +-0.054 is subnormal in e4m3)

_CACHE = {}


def _build(cfg=None):
    from contextlib import ExitStack

    from concourse import bacc
    import concourse.mybir as mybir
    import concourse.tile as tile

    cfg = cfg or {}
    FP32 = mybir.dt.float32
    F8 = mybir.dt.float8e4
    BF16 = mybir.dt.bfloat16
    DR = mybir.MatmulPerfMode.DoubleRow
    EXP = mybir.ActivationFunctionType.Exp
    COPY = mybir.ActivationFunctionType.Copy
    ADD = mybir.AluOpType.add

    nc = bacc.Bacc("TRN2", debug=False, num_devices=NCORES, dynamic_dma_scratch_size=4096)

    # X^T in permuted column order (core's own key tiles first), fp8 hi/lo.
    # layout [dpart, cc, di, col]: contraction index d = cc*256 + di*128 + dpart
    xh_d = nc.dram_tensor("xh", [128, 4, 2, 2048], F8, kind="ExternalInput").ap()
    xl_d = nc.dram_tensor("xl", [128, 4, 2, 2048], F8, kind="ExternalInput").ap()
    w_d = {}
    for wn in ("wk", "wq", "wv"):
        for part in ("h", "l"):
            w_d[wn + part] = nc.dram_tensor(
                wn + part, [128, 4, 2, 1024], F8, kind="ExternalInput"
            ).ap()
    # masks: [:,0:128] diag tri (0 if q>=k else -1e9); [:,128:256] phantom
    # (all 0 on even cores, all -1e9 on odd cores)
    mask_d = nc.dram_tensor("maskt", [128, 256], FP32, kind="ExternalInput").ap()
    ones_d = nc.dram_tensor("ones", [128, 1], BF16, kind="ExternalInput").ap()
    ob_d = nc.dram_tensor("ob", [NT, 128, D], BF16, kind="ExternalOutput").ap()
    rs_d = nc.dram_tensor("rs", [128, NT], FP32, kind="ExternalOutput").ap()

    INV = float(1.0 / WSC)

    with tile.TileContext(nc) as tc, ExitStack() as ctx:
        const = ctx.enter_context(tc.tile_pool(name="const", bufs=1))
        resident = ctx.enter_context(tc.tile_pool(name="resident", bufs=1))

        mask_sb = const.tile([128, 256], FP32)
        ones_sb = const.tile([128, 1], BF16)

        kt_sb = resident.tile([128, 8, 1024], BF16)   # K^T [dpart | c, own keys]
        qt_sb = resident.tile([128, 8, 2048], BF16)   # Q^T [dpart | c, all queries]
        v_sb = resident.tile([128, 8, 1024], BF16)    # V  [kpart | own ktile j, dv]

        # ---------------- Phase A: projections (fp8 DR, 3-term) ----------------
        # NOTE: GPSIMD/Pool cannot access PSUM on real HW -> DVE/Act only
        def psum_copy_scaled(i, dst, src):
            if i % 2 == 0:
                nc.scalar.activation(dst, src, COPY, scale=INV)
            else:
                nc.vector.tensor_scalar_mul(dst, src, INV)

        with tc.tile_pool(name="xpool", bufs=1) as xp, \
             tc.tile_pool(name="wpool", bufs=6) as wp, \
             tc.tile_pool(name="apsum", bufs=cfg.get("abufs", 6), space="PSUM") as aps:

            xh = xp.tile([128, 4, 2, 2048], F8)
            xl = xp.tile([128, 4, 2, 2048], F8)
            # chunked + multi-queue so the first K-proj tile can start ~3us in
            for c0 in range(0, 2048, 512):
                nc.sync.dma_start(xh[:, :, :, c0:c0 + 512], xh_d[:, :, :, c0:c0 + 512])
                nc.sync.dma_start(xl[:, :, :, c0:c0 + 512], xl_d[:, :, :, c0:c0 + 512])

            def load_w(name):
                h = wp.tile([128, 4, 2, 1024], F8, name=name + "h", tag="w")
                l = wp.tile([128, 4, 2, 1024], F8, name=name + "l", tag="w")
                q = nc.scalar
                if name == "wk":
                    for m0, m1 in ((0, 512), (512, 1024)):
                        q.dma_start(h[:, :, :, m0:m1], w_d[name + "h"][:, :, :, m0:m1])
                        q.dma_start(l[:, :, :, m0:m1], w_d[name + "l"][:, :, :, m0:m1])
                else:
                    q.dma_start(h[:], w_d[name + "h"])
                    q.dma_start(l[:], w_d[name + "l"])
                return h, l

            def proj_tile(ci, ps, wh, wl, xcols, m):
                """ps[128,512] += sum_cc (W^T X)[m-chunk, xcols] via 12 DR matmuls.

                hh terms first: the first tile can start before the lo
                tensors have even arrived from HBM."""
                terms = []
                for cc in range(4):
                    terms.append((wh[:, cc, :, m * 128:(m + 1) * 128], xh[:, cc, :, xcols]))
                for cc in range(4):
                    terms.append((wh[:, cc, :, m * 128:(m + 1) * 128], xl[:, cc, :, xcols]))
                for cc in range(4):
                    terms.append((wl[:, cc, :, m * 128:(m + 1) * 128], xh[:, cc, :, xcols]))
                for i, (lt, rt) in enumerate(terms):
                    nc.tensor.matmul(
                        ps[:], lt, rt,
                        start=(i == 0), stop=(i == len(terms) - 1),
                        perf_mode=DR,
                    )

            def vproj_tile(ps, xcols, wvh, wvl, h):
                terms = []
                for cc in range(4):
                    terms.append((xh[:, cc, :, xcols], wvh[:, cc, :, h * 512:(h + 1) * 512]))
                for cc in range(4):
                    terms.append((xh[:, cc, :, xcols], wvl[:, cc, :, h * 512:(h + 1) * 512]))
                for cc in range(4):
                    terms.append((xl[:, cc, :, xcols], wvh[:, cc, :, h * 512:(h + 1) * 512]))
                for i, (lt, rt) in enumerate(terms):
                    nc.tensor.matmul(
                        ps[:], lt, rt,
                        start=(i == 0), stop=(i == len(terms) - 1),
                        perf_mode=DR,
                    )

            ci = 0
            # K^T: own keys = first 1024 permuted columns
            wkh, wkl = load_w("wk")
            for kc in range(2):
                for m in range(8):
                    ps = aps.tile([128, 512], FP32, tag="ps")
                    proj_tile(ci, ps, wkh, wkl, slice(kc * 512, (kc + 1) * 512), m)
                    psum_copy_scaled(ci, kt_sb[:, m, kc * 512:(kc + 1) * 512], ps[:])
                    ci += 1
            # Q^T: all 2048 columns
            wqh, wql = load_w("wq")
            for qc in range(4):
                for m in range(8):
                    ps = aps.tile([128, 512], FP32, tag="ps")
                    proj_tile(ci, ps, wqh, wql, slice(qc * 512, (qc + 1) * 512), m)
                    psum_copy_scaled(ci, qt_sb[:, m, qc * 512:(qc + 1) * 512], ps[:])
                    ci += 1
            # V: own key tiles as stationary, W_V as moving
            wvh, wvl = load_w("wv")
            nc.scalar.dma_start(mask_sb[:], mask_d)
            nc.scalar.dma_start(ones_sb[:], ones_d)
            for j in range(8):
                for h in range(2):
                    ps = aps.tile([128, 512], FP32, tag="ps")
                    vproj_tile(ps, slice(j * 128, (j + 1) * 128), wvh, wvl, h)
                    psum_copy_scaled(ci, v_sb[:, j, h * 512:(h + 1) * 512], ps[:])
                    ci += 1

        # ---------------- Phase B1: S^T + exp -> P^T (bf16) ----------------
        # P^T[j]: [128 keys, own (8-j)*128 | other (8-j)*128] columns
        pt = []
        with tc.tile_pool(name="ptpool", bufs=1) as ptp:
            for j in range(8):
                pt.append(ptp.tile([128, 2 * (8 - j) * 128], BF16, name=f"pt{j}"))

            with tc.tile_pool(name="spsum", bufs=cfg.get("sbufs", 4), space="PSUM") as sps:
                for j in range(8):
                    seg = (8 - j) * 128
                    for half in range(2):      # 0 = own, 1 = other
                        q0 = j * 128 + half * 1024
                        off = half * seg
                        done = 0
                        while done < seg:
                            cw = min(512, seg - done)
                            ps = sps.tile([128, 512], FP32, tag="s")
                            for c in range(8):
                                nc.tensor.matmul(
                                    ps[:, 0:cw],
                                    kt_sb[:, c, j * 128:(j + 1) * 128],
                                    qt_sb[:, c, q0 + done:q0 + done + cw],
                                    start=(c == 0), stop=(c == 7),
                                )
                            if done == 0:
                                nc.vector.tensor_tensor(
                                    ps[:, 0:128], ps[:, 0:128],
                                    mask_sb[:, half * 128:half * 128 + 128], ADD,
                                )
                            nc.scalar.activation(
                                pt[j][:, off + done:off + done + cw],
                                ps[:, 0:cw], EXP, scale=float(SCALE),
                            )
                            done += cw

            # ---------------- Phase B2: PV + rowsums ----------------
            with tc.tile_pool(name="opsum", bufs=cfg.get("obufs", 3), space="PSUM") as ops, \
                 tc.tile_pool(name="rpsum", bufs=1, space="PSUM") as rps, \
                 tc.tile_pool(name="ostage", bufs=cfg.get("ostb", 6)) as osp, \
                 tc.tile_pool(name="rstage", bufs=1) as rsp:  # noqa: F841

                r_ps = rps.tile([128, NT], FP32)

                def pt_slice(ql, j):
                    seg = (8 - j) * 128
                    if ql < 8:
                        off = (ql - j) * 128
                    else:
                        off = seg + (ql - 8 - j) * 128
                    return pt[j][:, off:off + 128]

                for ql in range(NT):
                    # own half: ql = j' -> jlim = j'; other half: ql = 8+i -> jlim = i
                    jlim = ql if ql < 8 else ql - 8
                    o_ps = ops.tile([128, D], FP32, tag="o")
                    o_sb = osp.tile([128, D], BF16, tag="ob")
                    for j in range(jlim + 1):
                        nc.tensor.matmul(
                            r_ps[:, ql:ql + 1], pt_slice(ql, j), ones_sb[:],
                            start=(j == 0), stop=(j == jlim),
                            skip_group_check=True,
                        )
                    for h in range(2):
                        for j in range(jlim + 1):
                            nc.tensor.matmul(
                                o_ps[:, h * 512:(h + 1) * 512],
                                pt_slice(ql, j), v_sb[:, j, h * 512:(h + 1) * 512],
                                start=(j == 0), stop=(j == jlim),
                                skip_group_check=True,
                            )
                        # copy each 512-half as soon as its group stops; the
                        # other half's matmuls hide the copy+DMA latency
                        hs = slice(h * 512, (h + 1) * 512)
                        if (ql + h) % 2 == 0:
                            nc.scalar.activation(o_sb[:, hs], o_ps[:, hs], COPY)
                        else:
                            nc.vector.tensor_copy(o_sb[:, hs], o_ps[:, hs])
                        nc.sync.dma_start(ob_d[ql][:, hs], o_sb[:, hs])
                rs_sb = rsp.tile([128, NT], FP32)
                nc.vector.tensor_copy(rs_sb[:], r_ps[:])
                nc.scalar.dma_start(rs_d, rs_sb[:])

    nc.compile()
    return nc


def _make_runner(nc):
    """Cached jitted 8-core runner (no donation; avoids per-call re-jit)."""
    import jax
    import numpy as np_
    from jax.sharding import Mesh, PartitionSpec
    from jax.experimental.shard_map import shard_map

    from concourse import mybir
    from concourse.bass2jax import (
        _bass_exec_p,
        install_neuronx_cc_hook,
        partition_id_tensor,
    )

    install_neuronx_cc_hook()
    partition_name = nc.partition_id_tensor.name if nc.partition_id_tensor else None
    in_names, out_names, out_avals = [], [], []
    for alloc in nc.m.functions[0].allocations:
        if not isinstance(alloc, mybir.MemoryLocationSet):
            continue
        name = alloc.memorylocations[0].name
        if alloc.kind == "ExternalInput":
            if name != partition_name:
                in_names.append(name)
        elif alloc.kind == "ExternalOutput":
            out_names.append(name)
            out_avals.append(
                jax.core.ShapedArray(
                    tuple(alloc.tensor_shape), mybir.dt.np(alloc.dtype)
                )
            )
    n_params = len(in_names)
    all_in = list(in_names) + list(out_names)
    if partition_name is not None:
        all_in.append(partition_name)

    def _body(*args):
        operands = list(args)
        if partition_name is not None:
            operands.append(partition_id_tensor())
        return tuple(
            _bass_exec_p.bind(
                *operands,
                out_avals=tuple(out_avals),
                in_names=tuple(all_in),
                out_names=tuple(out_names),
                lowering_input_output_aliases=(),
                sim_require_finite=True,
                sim_require_nnan=True,
                nc=nc,
            )
        )

    devices = jax.devices()[:NCORES]
    mesh = Mesh(np_.asarray(devices), ("core",))
    spec = PartitionSpec("core")
    fn = jax.jit(
        shard_map(
            _body,
            mesh=mesh,
            in_specs=(spec,) * (n_params + len(out_names)),
            out_specs=(spec,) * len(out_names),
            check_rep=False,
        ),
        keep_unused=True,
    )

    def run(in_maps):
        concat_in = [
            np_.concatenate([np_.asarray(m[nm]) for m in in_maps], axis=0)
            for nm in in_names
        ]
        zeros = [
            np_.zeros((NCORES * a.shape[0], *a.shape[1:]), a.dtype) for a in out_avals
        ]
        outs = fn(*concat_in, *zeros)
        return [
            {
                nm: np_.asarray(outs[i]).reshape(NCORES, *out_avals[i].shape)[c]
                for i, nm in enumerate(out_names)
            }
            for c in range(NCORES)
        ]

    return run


def _perm(p):
    return [2 * j + p for j in range(8)] + [2 * i + (1 - p) for i in range(8)]


def _split_fp8(x):
    import ml_dtypes

    f8 = ml_dtypes.float8_e4m3
    h = x.astype(f8)
    l = (x - h.astype(np.float32)).astype(f8)
    return h, l


def _pack_dlayout(t):
    """[1024 d, N] fp32 -> hi/lo fp8 [128 dpart, 4 cc, 2 di, N]."""
    n = t.shape[1]
    r = np.ascontiguousarray(
        t.reshape(4, 2, 128, n).transpose(2, 0, 1, 3)
    )
    return _split_fp8(r)


def prepare_inputs(embeddings, W_Q, W_K, W_V):
    import ml_dtypes

    emb = np.asarray(embeddings, dtype=np.float32)
    ws = {}
    for name, w in (("wq", W_Q), ("wk", W_K), ("wv", W_V)):
        wh, wl = _pack_dlayout(np.asarray(w, dtype=np.float32) * np.float32(WSC))
        ws[name + "h"], ws[name + "l"] = wh, wl

    tri = (np.arange(128)[None, :] >= np.arange(128)[:, None]).astype(np.float32)
    neg = np.float32(-1e9)
    masks = []
    for p in range(2):
        m = np.zeros((128, 256), dtype=np.float32)
        m[:, 0:128] = np.where(tri > 0, 0.0, neg)   # diag: 0 iff q >= k
        m[:, 128:256] = 0.0 if p == 0 else neg      # phantom tile mask
        masks.append(m)
    ones_np = np.ones((128, 1), dtype=ml_dtypes.bfloat16)

    in_maps = []
    for core in range(NCORES):
        b, p = divmod(core, 2)
        x_t = emb[b].T  # [D, S]
        cols = np.concatenate([np.arange(g * 128, (g + 1) * 128) for g in _perm(p)])
        xp = np.ascontiguousarray(x_t[:, cols])
        xh, xl = _pack_dlayout(xp)
        in_maps.append(
            {
                "xh": xh, "xl": xl,
                "wqh": ws["wqh"], "wql": ws["wql"],
                "wkh": ws["wkh"], "wkl": ws["wkl"],
                "wvh": ws["wvh"], "wvl": ws["wvl"],
                "maskt": masks[p],
                "ones": ones_np,
            }
        )
    return in_maps


def merge_outputs(results):
    out = np.empty((B, S, D), dtype=np.float32)
    for b in range(B):
        osum = np.zeros((S, D), dtype=np.float32)
        rsum = np.zeros((S,), dtype=np.float32)
        for p in range(2):
            r = results[2 * b + p]
            ob = np.asarray(r["ob"]).astype(np.float32)   # [16,128,1024]
            rs = np.asarray(r["rs"]).astype(np.float32)   # [128,16]
            perm = _perm(p)
            for ql in range(NT):
                gq = perm[ql]
                osum[gq * 128:(gq + 1) * 128] += ob[ql]
                rsum[gq * 128:(gq + 1) * 128] += rs[:, ql]
        out[b] = osum / rsum[:, None]
    return out


def kernel(embeddings, W_Q, W_K, W_V):
    from concourse.bass_utils import run_bass_kernel_spmd

    if "nc" not in _CACHE:
        _CACHE["nc"] = _build()
    nc = _CACHE["nc"]

    in_maps = prepare_inputs(embeddings, W_Q, W_K, W_V)

    results = None
    try:
        if "runner" not in _CACHE:
            _CACHE["runner"] = _make_runner(nc)
        results = _CACHE["runner"](in_maps)
    except Exception:
        _CACHE.pop("runner", None)
    if results is None:
        import time as _time

        for attempt in range(2):
            try:
                results = run_bass_kernel_spmd(
                    nc, in_maps, core_ids=list(range(NCORES))
                ).results
                break
            except Exception:
                if attempt == 1:
                    raise
                _time.sleep(3.0)

    return merge_outputs(results)


# revision 3
# speedup vs baseline: 1.1162x; 1.0302x over previous
"""Causal single-head attention (B=4, S=2048, D=1024) on 8 Trainium2 cores.

Sharding: 2 cores per batch, interleaved KEY tiles (core parity p owns global
key tiles 2j+p). Each core computes K^T,V for its 1024 keys, Q^T for all 2048
queries, then S^T-major flash attention over its keys, producing UNNORMALIZED
partial outputs + partial rowsums. The host merges the two cores of a batch:
out = (Ohat_e + Ohat_o) / (rs_e + rs_o).  (No max-subtraction, like the
fp32r baseline; exp arguments are small enough.)

Numerics / speed:
  - Projections run as fp8(e4m3) DoubleRow matmuls with error compensation:
    X^T and 64*W are split on the HOST into hi + lo e4m3 parts and the three
    products hh, hl, lh are accumulated in fp32 PSUM (the tiny lo*lo term is
    dropped). DoubleRow contracts 256 elements per 0.5 cyc/row -> 3x fp32r
    throughput for ~1e-3 relative error. PSUM->SBUF copies rescale by 1/64.
  - Attention (S^T and PV) runs in bf16 (1 cyc/row, same as fp32r, but
    transpose-free): S^T = K^T.T Q^T is computed key-major so exp() writes
    P^T directly in the layout PV needs; rowsums come from an extra ap=1
    matmul against a ones vector reusing the PV stationary.
  - The program is SPMD-identical on all 8 cores; parity enters only through
    the host-packed column permutation of X^T and two mask tiles (diagonal
    tri mask + a phantom-tile mask that zeroes the odd core's extra tile).
"""

import numpy as np

B, S, D = 4, 2048, 1024
NCORES = 8
NT = 16             # 128-row tiles per sequence
SCALE = 1.0 / np.sqrt(np.float32(D))
# weight pre-scale for fp8 (W ~ +-0.054 is subnormal in e4m3). K^T/Q^T stay in
# x32-scaled form in SBUF (so hi/lo splitting is a plain copy + subtract and
# 32*|K| < 240 stays in e4m3 range); the 1/(32*32) comes out in the exp scale.
# V is unscaled during its PSUM copy.
WSC = 32.0

_CACHE = {}


def _build(cfg=None):
    from contextlib import ExitStack

    from concourse import bacc
    import concourse.mybir as mybir
    import concourse.tile as tile

    cfg = cfg or {}
    FP32 = mybir.dt.float32
    F8 = mybir.dt.float8e4
    BF16 = mybir.dt.bfloat16
    DR = mybir.MatmulPerfMode.DoubleRow
    EXP = mybir.ActivationFunctionType.Exp
    COPY = mybir.ActivationFunctionType.Copy
    ADD = mybir.AluOpType.add

    nc = bacc.Bacc("TRN2", debug=False, num_devices=NCORES, dynamic_dma_scratch_size=4096)

    # X^T in permuted column order (core's own key tiles first), fp8 hi/lo.
    # layout [dpart, cc, di, col]: contraction index d = cc*256 + di*128 + dpart
    xh_d = nc.dram_tensor("xh", [128, 4, 2, 2048], F8, kind="ExternalInput").ap()
    xl_d = nc.dram_tensor("xl", [128, 4, 2, 2048], F8, kind="ExternalInput").ap()
    w_d = {}
    for wn in ("wk", "wq", "wv"):
        for part in ("h", "l"):
            w_d[wn + part] = nc.dram_tensor(
                wn + part, [128, 4, 2, 1024], F8, kind="ExternalInput"
            ).ap()
    # masks: [:,0:128] diag tri (0 if q>=k else -1e9); [:,128:256] phantom
    # (all 0 on even cores, all -1e9 on odd cores)
    mask_d = nc.dram_tensor("maskt", [128, 256], FP32, kind="ExternalInput").ap()
    ones_d = nc.dram_tensor("ones", [128, 1], BF16, kind="ExternalInput").ap()
    ob_d = nc.dram_tensor("ob", [NT, 128, D], BF16, kind="ExternalOutput").ap()
    rs_d = nc.dram_tensor("rs", [128, NT], FP32, kind="ExternalOutput").ap()

    INV = float(1.0 / WSC)

    with tile.TileContext(nc) as tc, ExitStack() as ctx:
        const = ctx.enter_context(tc.tile_pool(name="const", bufs=1))
        resident = ctx.enter_context(tc.tile_pool(name="resident", bufs=1))

        mask_sb = const.tile([128, 256], FP32)
        ones_sb = const.tile([128, 1], BF16)

        # K^T/Q^T: x32-scaled fp8 hi/lo in DoubleRow layout [dpart, cc, di, col]
        kth = resident.tile([128, 4, 2, 1024], F8)
        ktl = resident.tile([128, 4, 2, 1024], F8)
        qth = resident.tile([128, 4, 2, 2048], F8)
        qtl = resident.tile([128, 4, 2, 2048], F8)
        v_sb = resident.tile([128, 8, 1024], BF16)    # V  [kpart | own ktile j, dv]

        # ---------------- Phase A: projections (fp8 DR, 3-term) ----------------
        # NOTE: GPSIMD/Pool cannot access PSUM on real HW -> DVE/Act only
        SUB = mybir.AluOpType.subtract

        def psum_split_f8(dsth, dstl, src):
            """hi = f8(psum); lo = f8(psum - hi). Keeps the x32 scale."""
            nc.scalar.activation(dsth, src, COPY)
            nc.vector.tensor_tensor(dstl, src, dsth, SUB)

        def psum_copy_scaled(i, dst, src):
            if i % 2 == 0:
                nc.scalar.activation(dst, src, COPY, scale=INV)
            else:
                nc.vector.tensor_scalar_mul(dst, src, INV)

        # PSUM plan (8 banks): apsum(4) for projections; spsum(2)+rpsum(1)
        # co-resident so B1 needs no pool barrier; opsum(2x2) replaces apsum
        # during B1 so B1->B2 needs no barrier either. Pools close LIFO, so
        # the long-lived spsum/rpsum open first.
        sps_cm = tc.tile_pool(name="spsum", bufs=cfg.get("sbufs", 3), space="PSUM")
        sps = sps_cm.__enter__()
        rps_cm = tc.tile_pool(name="rpsum", bufs=1, space="PSUM")
        rps = rps_cm.__enter__()
        aps_cm = tc.tile_pool(name="apsum", bufs=cfg.get("abufs", 4), space="PSUM")
        aps = aps_cm.__enter__()

        with tc.tile_pool(name="xpool", bufs=1) as xp, \
             tc.tile_pool(name="wpool", bufs=6) as wp:

            xh = xp.tile([128, 4, 2, 2048], F8)
            xl = xp.tile([128, 4, 2, 2048], F8)
            # chunked + multi-queue; first K-proj tile (hh terms) only needs
            # xh chunk 0 + wkh half 0, per-cc mini-chunks so the first DR
            # matmul can start ~3us in
            for cc in range(4):
                nc.sync.dma_start(xh[:, cc, :, 0:512], xh_d[:, cc, :, 0:512])

            def load_w(name):
                h = wp.tile([128, 4, 2, 1024], F8, name=name + "h", tag="w")
                l = wp.tile([128, 4, 2, 1024], F8, name=name + "l", tag="w")
                q = nc.scalar
                if name == "wk":
                    for cc in range(4):
                        q.dma_start(h[:, cc, :, 0:512], w_d[name + "h"][:, cc, :, 0:512])
                    q.dma_start(l[:, :, :, 0:512], w_d[name + "l"][:, :, :, 0:512])
                    q.dma_start(h[:, :, :, 512:1024], w_d[name + "h"][:, :, :, 512:1024])
                    q.dma_start(l[:, :, :, 512:1024], w_d[name + "l"][:, :, :, 512:1024])
                else:
                    q.dma_start(h[:], w_d[name + "h"])
                    q.dma_start(l[:], w_d[name + "l"])
                return h, l

            def proj_tile(ci, ps, wh, wl, xcols, m):
                """ps[128,512] += sum_cc (W^T X)[m-chunk, xcols] via 12 DR matmuls.

                hh terms first: the first tile can start before the lo
                tensors have even arrived from HBM."""
                terms = []
                for cc in range(4):
                    terms.append((wh[:, cc, :, m * 128:(m + 1) * 128], xh[:, cc, :, xcols]))
                for cc in range(4):
                    terms.append((wh[:, cc, :, m * 128:(m + 1) * 128], xl[:, cc, :, xcols]))
                for cc in range(4):
                    terms.append((wl[:, cc, :, m * 128:(m + 1) * 128], xh[:, cc, :, xcols]))
                for i, (lt, rt) in enumerate(terms):
                    nc.tensor.matmul(
                        ps[:], lt, rt,
                        start=(i == 0), stop=(i == len(terms) - 1),
                        perf_mode=DR,
                    )

            def vproj_tile(ps, xcols, wvh, wvl, h):
                terms = []
                for cc in range(4):
                    terms.append((xh[:, cc, :, xcols], wvh[:, cc, :, h * 512:(h + 1) * 512]))
                for cc in range(4):
                    terms.append((xh[:, cc, :, xcols], wvl[:, cc, :, h * 512:(h + 1) * 512]))
                for cc in range(4):
                    terms.append((xl[:, cc, :, xcols], wvh[:, cc, :, h * 512:(h + 1) * 512]))
                for i, (lt, rt) in enumerate(terms):
                    nc.tensor.matmul(
                        ps[:], lt, rt,
                        start=(i == 0), stop=(i == len(terms) - 1),
                        perf_mode=DR,
                    )

            ci = 0
            # K^T: own keys = first 1024 permuted columns
            wkh, wkl = load_w("wk")
            # remaining X chunks split across BOTH queues so the W streams
            # don't starve K-proj of its x-chunks on the shared DMA engines
            nc.sync.dma_start(xl[:, :, :, 0:512], xl_d[:, :, :, 0:512])
            nc.sync.dma_start(xh[:, :, :, 512:1024], xh_d[:, :, :, 512:1024])
            nc.scalar.dma_start(xl[:, :, :, 512:1024], xl_d[:, :, :, 512:1024])
            nc.sync.dma_start(xh[:, :, :, 1024:1536], xh_d[:, :, :, 1024:1536])
            nc.scalar.dma_start(xl[:, :, :, 1024:1536], xl_d[:, :, :, 1024:1536])
            nc.sync.dma_start(xh[:, :, :, 1536:2048], xh_d[:, :, :, 1536:2048])
            nc.sync.dma_start(xl[:, :, :, 1536:2048], xl_d[:, :, :, 1536:2048])
            for kc in range(2):
                for m in range(8):
                    ps = aps.tile([128, 512], FP32, tag="ps")
                    proj_tile(ci, ps, wkh, wkl, slice(kc * 512, (kc + 1) * 512), m)
                    ks = (slice(None), m // 2, m % 2, slice(kc * 512, (kc + 1) * 512))
                    psum_split_f8(kth[ks], ktl[ks], ps[:])
                    ci += 1
            # Q^T: all 2048 columns
            wqh, wql = load_w("wq")
            for qc in range(4):
                for m in range(8):
                    ps = aps.tile([128, 512], FP32, tag="ps")
                    proj_tile(ci, ps, wqh, wql, slice(qc * 512, (qc + 1) * 512), m)
                    qs = (slice(None), m // 2, m % 2, slice(qc * 512, (qc + 1) * 512))
                    psum_split_f8(qth[qs], qtl[qs], ps[:])
                    ci += 1
            # V: own key tiles as stationary, W_V as moving
            wvh, wvl = load_w("wv")
            nc.scalar.dma_start(mask_sb[:], mask_d)
            nc.scalar.dma_start(ones_sb[:], ones_d)
            for j in range(8):
                for h in range(2):
                    ps = aps.tile([128, 512], FP32, tag="ps")
                    vproj_tile(ps, slice(j * 128, (j + 1) * 128), wvh, wvl, h)
                    psum_copy_scaled(ci, v_sb[:, j, h * 512:(h + 1) * 512], ps[:])
                    ci += 1

        # ---------------- Phase B1: S^T + exp -> P^T (bf16) ----------------
        # P^T[j]: [128 keys, own (8-j)*128 | other (8-j)*128] columns
        aps_cm.__exit__(None, None, None)

        pt = []
        with tc.tile_pool(name="ptpool", bufs=1) as ptp:
            for j in range(8):
                pt.append(ptp.tile([128, 2 * (8 - j) * 128], BF16, name=f"pt{j}"))

            if True:
                for j in range(8):
                    seg = (8 - j) * 128
                    for half in range(2):      # 0 = own, 1 = other
                        q0 = j * 128 + half * 1024
                        off = half * seg
                        done = 0
                        while done < seg:
                            cw = min(512, seg - done)
                            ps = sps.tile([128, 512], FP32, tag="s")
                            kcols = slice(j * 128, (j + 1) * 128)
                            qcols = slice(q0 + done, q0 + done + cw)
                            terms = []
                            for cc in range(4):
                                terms.append((kth[:, cc, :, kcols], qth[:, cc, :, qcols]))
                            for cc in range(4):
                                terms.append((kth[:, cc, :, kcols], qtl[:, cc, :, qcols]))
                            for cc in range(4):
                                terms.append((ktl[:, cc, :, kcols], qth[:, cc, :, qcols]))
                            for i, (lt, rt) in enumerate(terms):
                                nc.tensor.matmul(
                                    ps[:, 0:cw], lt, rt,
                                    start=(i == 0), stop=(i == len(terms) - 1),
                                    perf_mode=DR,
                                )
                            if done == 0:
                                nc.vector.tensor_tensor(
                                    ps[:, 0:128], ps[:, 0:128],
                                    mask_sb[:, half * 128:half * 128 + 128], ADD,
                                )
                            nc.scalar.activation(
                                pt[j][:, off + done:off + done + cw],
                                ps[:, 0:cw], EXP, scale=float(SCALE / (WSC * WSC)),
                            )
                            done += cw

            # ---------------- Phase B2: PV + rowsums ----------------
            # O accumulates per 512-half in single-bank tiles: 5 bufs = 2.5
            # qtiles in flight within the 8-bank budget (3 spsum + 1 rpsum)
            with tc.tile_pool(name="opsum", bufs=cfg.get("obufs", 4), space="PSUM") as ops, \
                 tc.tile_pool(name="ostage", bufs=cfg.get("ostb", 6)) as osp, \
                 tc.tile_pool(name="rstage", bufs=1) as rsp:  # noqa: F841

                r_ps = rps.tile([128, NT], FP32)

                def pt_slice(ql, j):
                    seg = (8 - j) * 128
                    if ql < 8:
                        off = (ql - j) * 128
                    else:
                        off = seg + (ql - 8 - j) * 128
                    return pt[j][:, off:off + 128]

                # end on the second-biggest tile: ql=14's matmuls hide ql=15's
                # copy+DMA drain, leaving only one tile's tail exposed
                for ql in list(range(14)) + [15, 14]:
                    # own half: ql = j' -> jlim = j'; other half: ql = 8+i -> jlim = i
                    jlim = ql if ql < 8 else ql - 8
                    o_sb = osp.tile([128, D], BF16, tag="ob")
                    for j in range(jlim + 1):
                        nc.tensor.matmul(
                            r_ps[:, ql:ql + 1], pt_slice(ql, j), ones_sb[:],
                            start=(j == 0), stop=(j == jlim),
                            skip_group_check=True,
                        )
                    for h in range(2):
                        o_ps = ops.tile([128, 512], FP32, tag="o")
                        for j in range(jlim + 1):
                            nc.tensor.matmul(
                                o_ps[:],
                                pt_slice(ql, j), v_sb[:, j, h * 512:(h + 1) * 512],
                                start=(j == 0), stop=(j == jlim),
                                skip_group_check=True,
                            )
                        # copy each 512-half as soon as its group stops; the
                        # other half's matmuls hide the copy+DMA latency
                        hs = slice(h * 512, (h + 1) * 512)
                        if (ql + h) % 2 == 0:
                            nc.scalar.activation(o_sb[:, hs], o_ps[:], COPY)
                        else:
                            nc.vector.tensor_copy(o_sb[:, hs], o_ps[:])
                        nc.sync.dma_start(ob_d[ql][:, hs], o_sb[:, hs])
                rs_sb = rsp.tile([128, NT], FP32)
                nc.vector.tensor_copy(rs_sb[:], r_ps[:])
                nc.scalar.dma_start(rs_d, rs_sb[:])

        rps_cm.__exit__(None, None, None)
        sps_cm.__exit__(None, None, None)

    nc.compile()
    return nc


def _make_runner(nc):
    """Cached jitted 8-core runner (no donation; avoids per-call re-jit)."""
    import jax
    import numpy as np_
    from jax.sharding import Mesh, PartitionSpec
    from jax.experimental.shard_map import shard_map

    from concourse import mybir
    from concourse.bass2jax import (
        _bass_exec_p,
        install_neuronx_cc_hook,
        partition_id_tensor,
    )

    install_neuronx_cc_hook()
    partition_name = nc.partition_id_tensor.name if nc.partition_id_tensor else None
    in_names, out_names, out_avals = [], [], []
    for alloc in nc.m.functions[0].allocations:
        if not isinstance(alloc, mybir.MemoryLocationSet):
            continue
        name = alloc.memorylocations[0].name
        if alloc.kind == "ExternalInput":
            if name != partition_name:
                in_names.append(name)
        elif alloc.kind == "ExternalOutput":
            out_names.append(name)
            out_avals.append(
                jax.core.ShapedArray(
                    tuple(alloc.tensor_shape), mybir.dt.np(alloc.dtype)
                )
            )
    n_params = len(in_names)
    all_in = list(in_names) + list(out_names)
    if partition_name is not None:
        all_in.append(partition_name)

    def _body(*args):
        operands = list(args)
        if partition_name is not None:
            operands.append(partition_id_tensor())
        return tuple(
            _bass_exec_p.bind(
                *operands,
                out_avals=tuple(out_avals),
                in_names=tuple(all_in),
                out_names=tuple(out_names),
                lowering_input_output_aliases=(),
                sim_require_finite=True,
                sim_require_nnan=True,
                nc=nc,
            )
        )

    devices = jax.devices()[:NCORES]
    mesh = Mesh(np_.asarray(devices), ("core",))
    spec = PartitionSpec("core")
    fn = jax.jit(
        shard_map(
            _body,
            mesh=mesh,
            in_specs=(spec,) * (n_params + len(out_names)),
            out_specs=(spec,) * len(out_names),
            check_rep=False,
        ),
        keep_unused=True,
    )

    def run(in_maps):
        concat_in = [
            np_.concatenate([np_.asarray(m[nm]) for m in in_maps], axis=0)
            for nm in in_names
        ]
        zeros = [
            np_.zeros((NCORES * a.shape[0], *a.shape[1:]), a.dtype) for a in out_avals
        ]
        outs = fn(*concat_in, *zeros)
        return [
            {
                nm: np_.asarray(outs[i]).reshape(NCORES, *out_avals[i].shape)[c]
                for i, nm in enumerate(out_names)
            }
            for c in range(NCORES)
        ]

    return run


def _perm(p):
    return [2 * j + p for j in range(8)] + [2 * i + (1 - p) for i in range(8)]


def _split_fp8(x):
    import ml_dtypes

    f8 = ml_dtypes.float8_e4m3
    h = x.astype(f8)
    l = (x - h.astype(np.float32)).astype(f8)
    return h, l


def _pack_dlayout(t):
    """[1024 d, N] fp32 -> hi/lo fp8 [128 dpart, 4 cc, 2 di, N]."""
    n = t.shape[1]
    r = np.ascontiguousarray(
        t.reshape(4, 2, 128, n).transpose(2, 0, 1, 3)
    )
    return _split_fp8(r)


def prepare_inputs(embeddings, W_Q, W_K, W_V):
    import ml_dtypes

    emb = np.asarray(embeddings, dtype=np.float32)
    ws = {}
    for name, w in (("wq", W_Q), ("wk", W_K), ("wv", W_V)):
        wh, wl = _pack_dlayout(np.asarray(w, dtype=np.float32) * np.float32(WSC))
        ws[name + "h"], ws[name + "l"] = wh, wl

    tri = (np.arange(128)[None, :] >= np.arange(128)[:, None]).astype(np.float32)
    neg = np.float32(-1e9)
    masks = []
    for p in range(2):
        m = np.zeros((128, 256), dtype=np.float32)
        m[:, 0:128] = np.where(tri > 0, 0.0, neg)   # diag: 0 iff q >= k
        m[:, 128:256] = 0.0 if p == 0 else neg      # phantom tile mask
        masks.append(m)
    ones_np = np.ones((128, 1), dtype=ml_dtypes.bfloat16)

    in_maps = []
    for core in range(NCORES):
        b, p = divmod(core, 2)
        x_t = emb[b].T  # [D, S]
        cols = np.concatenate([np.arange(g * 128, (g + 1) * 128) for g in _perm(p)])
        xp = np.ascontiguousarray(x_t[:, cols])
        xh, xl = _pack_dlayout(xp)
        in_maps.append(
            {
                "xh": xh, "xl": xl,
                "wqh": ws["wqh"], "wql": ws["wql"],
                "wkh": ws["wkh"], "wkl": ws["wkl"],
                "wvh": ws["wvh"], "wvl": ws["wvl"],
                "maskt": masks[p],
                "ones": ones_np,
            }
        )
    return in_maps


def merge_outputs(results):
    out = np.empty((B, S, D), dtype=np.float32)
    for b in range(B):
        osum = np.zeros((S, D), dtype=np.float32)
        rsum = np.zeros((S,), dtype=np.float32)
        for p in range(2):
            r = results[2 * b + p]
            ob = np.asarray(r["ob"]).astype(np.float32)   # [16,128,1024]
            rs = np.asarray(r["rs"]).astype(np.float32)   # [128,16]
            perm = _perm(p)
            for ql in range(NT):
                gq = perm[ql]
                osum[gq * 128:(gq + 1) * 128] += ob[ql]
                rsum[gq * 128:(gq + 1) * 128] += rs[:, ql]
        out[b] = osum / rsum[:, None]
    return out


def kernel(embeddings, W_Q, W_K, W_V):
    from concourse.bass_utils import run_bass_kernel_spmd

    if "nc" not in _CACHE:
        _CACHE["nc"] = _build()
    nc = _CACHE["nc"]

    in_maps = prepare_inputs(embeddings, W_Q, W_K, W_V)

    results = None
    try:
        if "runner" not in _CACHE:
            _CACHE["runner"] = _make_runner(nc)
        results = _CACHE["runner"](in_maps)
    except Exception:
        _CACHE.pop("runner", None)
    if results is None:
        import time as _time

        for attempt in range(2):
            try:
                results = run_bass_kernel_spmd(
                    nc, in_maps, core_ids=list(range(NCORES))
                ).results
                break
            except Exception:
                if attempt == 1:
                    raise
                _time.sleep(3.0)

    return merge_outputs(results)


# revision 4
# speedup vs baseline: 1.1317x; 1.0139x over previous
"""Causal single-head attention (B=4, S=2048, D=1024) on 8 Trainium2 cores.

Sharding: 2 cores per batch, interleaved KEY tiles (core parity p owns global
key tiles 2j+p). Each core computes K^T,V for its 1024 keys, Q^T for all 2048
queries, then S^T-major flash attention over its keys, producing UNNORMALIZED
partial outputs + partial rowsums. The host merges the two cores of a batch:
out = (Ohat_e + Ohat_o) / (rs_e + rs_o).  (No max-subtraction, like the
fp32r baseline; exp arguments are small enough.)

Numerics / speed:
  - Projections run as fp8(e4m3) DoubleRow matmuls with error compensation:
    X^T and 64*W are split on the HOST into hi + lo e4m3 parts and the three
    products hh, hl, lh are accumulated in fp32 PSUM (the tiny lo*lo term is
    dropped). DoubleRow contracts 256 elements per 0.5 cyc/row -> 3x fp32r
    throughput for ~1e-3 relative error. PSUM->SBUF copies rescale by 1/64.
  - Attention (S^T and PV) runs in bf16 (1 cyc/row, same as fp32r, but
    transpose-free): S^T = K^T.T Q^T is computed key-major so exp() writes
    P^T directly in the layout PV needs; rowsums come from an extra ap=1
    matmul against a ones vector reusing the PV stationary.
  - The program is SPMD-identical on all 8 cores; parity enters only through
    the host-packed column permutation of X^T and two mask tiles (diagonal
    tri mask + a phantom-tile mask that zeroes the odd core's extra tile).
"""

import numpy as np

B, S, D = 4, 2048, 1024
NCORES = 8
NT = 16             # 128-row tiles per sequence
SCALE = 1.0 / np.sqrt(np.float32(D))
# weight pre-scale for fp8 (W ~ +-0.054 is subnormal in e4m3). K^T/Q^T stay in
# x32-scaled form in SBUF (so hi/lo splitting is a plain copy + subtract and
# 32*|K| < 240 stays in e4m3 range); the 1/(32*32) comes out in the exp scale.
# V is unscaled during its PSUM copy.
WSC = 32.0

_CACHE = {}


def _build(cfg=None):
    from contextlib import ExitStack

    from concourse import bacc
    import concourse.mybir as mybir
    import concourse.tile as tile

    cfg = cfg or {}
    FP32 = mybir.dt.float32
    F8 = mybir.dt.float8e4
    BF16 = mybir.dt.bfloat16
    DR = mybir.MatmulPerfMode.DoubleRow
    EXP = mybir.ActivationFunctionType.Exp
    COPY = mybir.ActivationFunctionType.Copy
    ADD = mybir.AluOpType.add
    MULT = mybir.AluOpType.mult

    nc = bacc.Bacc("TRN2", debug=False, num_devices=NCORES, dynamic_dma_scratch_size=4096)

    # X^T in permuted column order (core's own key tiles first), fp8 hi/lo.
    # layout [dpart, cc, di, col]: contraction index d = cc*256 + di*128 + dpart
    xh_d = nc.dram_tensor("xh", [128, 4, 2, 2048], F8, kind="ExternalInput").ap()
    xl_d = nc.dram_tensor("xl", [128, 4, 2, 2048], F8, kind="ExternalInput").ap()
    w_d = {}
    for wn in ("wk", "wq", "wv"):
        for part in ("h", "l"):
            w_d[wn + part] = nc.dram_tensor(
                wn + part, [128, 4, 2, 1024], F8, kind="ExternalInput"
            ).ap()
    # masks: [:,0:128] diag tri (0 if q>=k else -1e9); [:,128:256] phantom
    # (all 0 on even cores, all -1e9 on odd cores)
    mask_d = nc.dram_tensor("maskt", [128, 256], FP32, kind="ExternalInput").ap()
    ones_d = nc.dram_tensor("ones", [128, 2, 1], F8, kind="ExternalInput").ap()
    ob_d = nc.dram_tensor("ob", [NT, 128, D], BF16, kind="ExternalOutput").ap()
    rs_d = nc.dram_tensor("rs", [128, NT], FP32, kind="ExternalOutput").ap()

    INV = float(1.0 / WSC)

    with tile.TileContext(nc) as tc, ExitStack() as ctx:
        const = ctx.enter_context(tc.tile_pool(name="const", bufs=1))
        resident = ctx.enter_context(tc.tile_pool(name="resident", bufs=1))

        mask_sb = const.tile([128, 256], FP32)
        ones_sb = const.tile([128, 2, 1], F8)
        nln4_sb = const.tile([128, 1], FP32)
        nc.gpsimd.memset(nln4_sb[:], -1.3862943611198906)

        # K^T/Q^T: x32-scaled fp8 hi/lo in DoubleRow layout [dpart, cc, di, col]
        kth = resident.tile([128, 4, 2, 1024], F8)
        ktl = resident.tile([128, 4, 2, 1024], F8)
        qth = resident.tile([128, 4, 2, 2048], F8)
        qtl = resident.tile([128, 4, 2, 2048], F8)
        # V: x32-scaled fp8 hi/lo in pair layout [kpart, pair a, sub, dv]
        vh = resident.tile([128, 4, 2, 1024], F8)
        vl = resident.tile([128, 4, 2, 1024], F8)

        # ---------------- Phase A: projections (fp8 DR, 3-term) ----------------
        # NOTE: GPSIMD/Pool cannot access PSUM on real HW -> DVE/Act only
        SUB = mybir.AluOpType.subtract

        def psum_split_f8(dsth, dstl, src):
            """hi = f8(psum); lo = f8(psum - hi). Keeps the x32 scale."""
            nc.scalar.activation(dsth, src, COPY)
            nc.vector.tensor_tensor(dstl, src, dsth, SUB)

        def psum_copy_scaled(i, dst, src):
            if i % 2 == 0:
                nc.scalar.activation(dst, src, COPY, scale=INV)
            else:
                nc.vector.tensor_scalar_mul(dst, src, INV)

        # PSUM plan (8 banks): apsum(4) for projections; spsum(2)+rpsum(1)
        # co-resident so B1 needs no pool barrier; opsum(2x2) replaces apsum
        # during B1 so B1->B2 needs no barrier either. Pools close LIFO, so
        # the long-lived spsum/rpsum open first.
        sps_cm = tc.tile_pool(name="spsum", bufs=cfg.get("sbufs", 3), space="PSUM")
        sps = sps_cm.__enter__()
        rps_cm = tc.tile_pool(name="rpsum", bufs=1, space="PSUM")
        rps = rps_cm.__enter__()
        aps_cm = tc.tile_pool(name="apsum", bufs=cfg.get("abufs", 4), space="PSUM")
        aps = aps_cm.__enter__()

        with tc.tile_pool(name="xpool", bufs=1) as xp, \
             tc.tile_pool(name="wpool", bufs=6) as wp:

            xh = xp.tile([128, 4, 2, 2048], F8)
            xl = xp.tile([128, 4, 2, 2048], F8)
            # chunked + multi-queue; first K-proj tile (hh terms) only needs
            # xh chunk 0 + wkh half 0, per-cc mini-chunks so the first DR
            # matmul can start ~3us in
            for cc in range(4):
                nc.sync.dma_start(xh[:, cc, :, 0:512], xh_d[:, cc, :, 0:512])

            def load_w(name):
                h = wp.tile([128, 4, 2, 1024], F8, name=name + "h", tag="w")
                l = wp.tile([128, 4, 2, 1024], F8, name=name + "l", tag="w")
                q = nc.scalar
                if name == "wk":
                    for cc in range(4):
                        q.dma_start(h[:, cc, :, 0:512], w_d[name + "h"][:, cc, :, 0:512])
                    q.dma_start(l[:, :, :, 0:512], w_d[name + "l"][:, :, :, 0:512])
                    q.dma_start(h[:, :, :, 512:1024], w_d[name + "h"][:, :, :, 512:1024])
                    q.dma_start(l[:, :, :, 512:1024], w_d[name + "l"][:, :, :, 512:1024])
                else:
                    q.dma_start(h[:], w_d[name + "h"])
                    q.dma_start(l[:], w_d[name + "l"])
                return h, l

            def proj_tile(ci, ps, wh, wl, xcols, m):
                """ps[128,512] += sum_cc (W^T X)[m-chunk, xcols] via 12 DR matmuls.

                hh terms first: the first tile can start before the lo
                tensors have even arrived from HBM."""
                terms = []
                for cc in range(4):
                    terms.append((wh[:, cc, :, m * 128:(m + 1) * 128], xh[:, cc, :, xcols]))
                for cc in range(4):
                    terms.append((wh[:, cc, :, m * 128:(m + 1) * 128], xl[:, cc, :, xcols]))
                for cc in range(4):
                    terms.append((wl[:, cc, :, m * 128:(m + 1) * 128], xh[:, cc, :, xcols]))
                for i, (lt, rt) in enumerate(terms):
                    nc.tensor.matmul(
                        ps[:], lt, rt,
                        start=(i == 0), stop=(i == len(terms) - 1),
                        perf_mode=DR,
                    )

            def vproj_tile(ps, xcols, wvh, wvl, h):
                terms = []
                for cc in range(4):
                    terms.append((xh[:, cc, :, xcols], wvh[:, cc, :, h * 512:(h + 1) * 512]))
                for cc in range(4):
                    terms.append((xh[:, cc, :, xcols], wvl[:, cc, :, h * 512:(h + 1) * 512]))
                for cc in range(4):
                    terms.append((xl[:, cc, :, xcols], wvh[:, cc, :, h * 512:(h + 1) * 512]))
                for i, (lt, rt) in enumerate(terms):
                    nc.tensor.matmul(
                        ps[:], lt, rt,
                        start=(i == 0), stop=(i == len(terms) - 1),
                        perf_mode=DR,
                    )

            ci = 0
            # K^T: own keys = first 1024 permuted columns
            wkh, wkl = load_w("wk")
            # remaining X chunks split across BOTH queues so the W streams
            # don't starve K-proj of its x-chunks on the shared DMA engines
            nc.sync.dma_start(xl[:, :, :, 0:512], xl_d[:, :, :, 0:512])
            nc.sync.dma_start(xh[:, :, :, 512:1024], xh_d[:, :, :, 512:1024])
            nc.scalar.dma_start(xl[:, :, :, 512:1024], xl_d[:, :, :, 512:1024])
            nc.sync.dma_start(xh[:, :, :, 1024:1536], xh_d[:, :, :, 1024:1536])
            nc.scalar.dma_start(xl[:, :, :, 1024:1536], xl_d[:, :, :, 1024:1536])
            nc.sync.dma_start(xh[:, :, :, 1536:2048], xh_d[:, :, :, 1536:2048])
            nc.sync.dma_start(xl[:, :, :, 1536:2048], xl_d[:, :, :, 1536:2048])
            for kc in range(2):
                for m in range(8):
                    ps = aps.tile([128, 512], FP32, tag="ps")
                    proj_tile(ci, ps, wkh, wkl, slice(kc * 512, (kc + 1) * 512), m)
                    ks = (slice(None), m // 2, m % 2, slice(kc * 512, (kc + 1) * 512))
                    psum_split_f8(kth[ks], ktl[ks], ps[:])
                    ci += 1
            # Q^T: all 2048 columns
            wqh, wql = load_w("wq")
            for qc in range(4):
                for m in range(8):
                    ps = aps.tile([128, 512], FP32, tag="ps")
                    proj_tile(ci, ps, wqh, wql, slice(qc * 512, (qc + 1) * 512), m)
                    qs = (slice(None), m // 2, m % 2, slice(qc * 512, (qc + 1) * 512))
                    psum_split_f8(qth[qs], qtl[qs], ps[:])
                    ci += 1
            # V: own key tiles as stationary, W_V as moving
            wvh, wvl = load_w("wv")
            nc.scalar.dma_start(mask_sb[:], mask_d)
            nc.scalar.dma_start(ones_sb[:], ones_d)
            for j in range(8):
                for h in range(2):
                    ps = aps.tile([128, 512], FP32, tag="ps")
                    vproj_tile(ps, slice(j * 128, (j + 1) * 128), wvh, wvl, h)
                    vs = (slice(None), j // 2, j % 2, slice(h * 512, (h + 1) * 512))
                    psum_split_f8(vh[vs], vl[vs], ps[:])
                    ci += 1

        # ---------------- Phase B1: S^T + exp -> P^T (bf16) ----------------
        # P^T[j]: [128 keys, own (8-j)*128 | other (8-j)*128] columns
        aps_cm.__exit__(None, None, None)

        # P^T pair tiles (fp8 hi/lo): pair a covers kts j=2a (sub 0) and
        # j=2a+1 (sub 1, shifted one qtile; its two leading 128-col regions
        # are zeroed so DoubleRow PV over the pair is uniformly correct).
        LN4 = 1.3862943611198906
        pth, ptl = [], []
        with tc.tile_pool(name="ptpool", bufs=1) as ptp, \
             tc.tile_pool(name="ptmp", bufs=6) as ptmpp:
            for a in range(4):
                nqa = 2 * (8 - 2 * a) * 128
                pth.append(ptp.tile([128, 2, nqa], F8, name=f"pth{a}"))
                ptl.append(ptp.tile([128, 2, nqa], F8, name=f"ptl{a}"))
            for a in range(4):
                La = (8 - 2 * a) * 128
                for t in (pth[a], ptl[a]):
                    nc.gpsimd.memset(t[:, 1, 0:128], 0.0)
                    nc.gpsimd.memset(t[:, 1, La:La + 128], 0.0)

            # B2 pools co-resident with B1's: spsum(3)+rpsum(1)+opsum(4) = 8
            with tc.tile_pool(name="opsum", bufs=cfg.get("obufs", 4), space="PSUM") as ops, \
                 tc.tile_pool(name="ostage", bufs=cfg.get("ostb", 6)) as osp, \
                 tc.tile_pool(name="rstage", bufs=1) as rsp:  # noqa: F841

                r_ps = rps.tile([128, NT], FP32)

                def emit_b1(j):
                    a, sub = j // 2, j % 2
                    La = (8 - 2 * a) * 128
                    seg = (8 - j) * 128
                    for half in range(2):      # 0 = own, 1 = other
                        q0 = j * 128 + half * 1024
                        off = half * La + sub * 128
                        done = 0
                        while done < seg:
                            cw = min(512, seg - done)
                            ps = sps.tile([128, 512], FP32, tag="s")
                            kcols = slice(j * 128, (j + 1) * 128)
                            qcols = slice(q0 + done, q0 + done + cw)
                            terms = []
                            for cc in range(4):
                                terms.append((kth[:, cc, :, kcols], qth[:, cc, :, qcols]))
                            for cc in range(4):
                                terms.append((kth[:, cc, :, kcols], qtl[:, cc, :, qcols]))
                            for cc in range(4):
                                terms.append((ktl[:, cc, :, kcols], qth[:, cc, :, qcols]))
                            for i, (lt, rt) in enumerate(terms):
                                nc.tensor.matmul(
                                    ps[:, 0:cw], lt, rt,
                                    start=(i == 0), stop=(i == len(terms) - 1),
                                    perf_mode=DR,
                                )
                            # 2-pass: exp -> fp32 tmp (Act); 0/1 mask on the
                            # SBUF tmp (DVE, off the PSUM critical path);
                            # hi on Pool, lo on DVE
                            ptmp = ptmpp.tile([128, 512], FP32, tag="pt")
                            nc.scalar.activation(
                                ptmp[:, 0:cw], ps[:, 0:cw], EXP,
                                scale=float(SCALE / (WSC * WSC)), bias=nln4_sb[:],
                            )
                            if done == 0:
                                nc.vector.tensor_tensor(
                                    ptmp[:, 0:128], ptmp[:, 0:128],
                                    mask_sb[:, half * 128:half * 128 + 128], MULT,
                                )
                            dsth = pth[a][:, sub, off + done:off + done + cw]
                            dstl = ptl[a][:, sub, off + done:off + done + cw]
                            nc.gpsimd.tensor_copy(dsth, ptmp[:, 0:cw])
                            nc.vector.tensor_tensor(dstl, ptmp[:, 0:cw], dsth, SUB)
                            done += cw

                def pv_col(ql, a):
                    La = (8 - 2 * a) * 128
                    if ql < 8:
                        return (ql - 2 * a) * 128
                    return La + (ql - 8 - 2 * a) * 128

                def emit_b2(ql, tail=False):
                    # own half: ql = j' -> jlim = j'; other: ql = 8+i -> jlim = i
                    jlim = ql if ql < 8 else ql - 8
                    alim = jlim // 2
                    o_sb = osp.tile([128, D], BF16, tag="ob")
                    for a in range(alim + 1):
                        c = pv_col(ql, a)
                        for i, pt_ in enumerate((pth[a], ptl[a])):
                            nc.tensor.matmul(
                                r_ps[:, ql:ql + 1], pt_[:, :, c:c + 128], ones_sb[:],
                                start=(a == 0 and i == 0), stop=(a == alim and i == 1),
                                perf_mode=DR, skip_group_check=True,
                            )
                    for h in range(2):
                        o_ps = ops.tile([128, 512], FP32, tag="o")
                        hs = slice(h * 512, (h + 1) * 512)
                        nterm = 3 * (alim + 1)
                        i = 0
                        for a in range(alim + 1):
                            c = pv_col(ql, a)
                            for pt_, v_ in ((pth[a], vh), (pth[a], vl), (ptl[a], vh)):
                                nc.tensor.matmul(
                                    o_ps[:],
                                    pt_[:, :, c:c + 128], v_[:, a, :, hs],
                                    start=(i == 0), stop=(i == nterm - 1),
                                    perf_mode=DR, skip_group_check=True,
                                )
                                i += 1
                        # copy each 512-half as soon as its group stops (and
                        # undo the x32 V scale); the other half's matmuls hide
                        # the copy+DMA latency
                        if tail and h == 1:
                            # final tile: quarter-split so copy/DMA pipeline
                            # and the exposed tail is one quarter, not a half
                            for q4 in range(4):
                                qs = slice(q4 * 128, (q4 + 1) * 128)
                                gs = slice(h * 512 + q4 * 128, h * 512 + (q4 + 1) * 128)
                                if q4 % 2 == 0:
                                    nc.scalar.activation(o_sb[:, gs], o_ps[:, qs], COPY, scale=INV)
                                else:
                                    nc.vector.tensor_scalar_mul(o_sb[:, gs], o_ps[:, qs], INV)
                                nc.sync.dma_start(ob_d[ql][:, gs], o_sb[:, gs])
                        elif (ql + h) % 2 == 0:
                            nc.scalar.activation(o_sb[:, hs], o_ps[:], COPY, scale=INV)
                            nc.sync.dma_start(ob_d[ql][:, hs], o_sb[:, hs])
                        else:
                            nc.vector.tensor_scalar_mul(o_sb[:, hs], o_ps[:], INV)
                            nc.sync.dma_start(ob_d[ql][:, hs], o_sb[:, hs])

                # interleave: each P^T pair a finalizes after B1 j=2a,2a+1, at
                # which point the B2 qtiles needing only pairs <= a can run.
                # Kills the B1->B2 phase boundary and spreads the copy load.
                # each wave's B2 runs one B1 iteration late so the exp->
                # hi/lo chains of its pair are fully drained when it starts
                waves = [[0, 1, 8, 9], [2, 3, 10, 11], [4, 5, 12, 13], [6, 7, 15, 14]]
                sched = [("b1", 0), ("b1", 1), ("b1", 2), ("w", 0),
                         ("b1", 3), ("b1", 4), ("w", 1),
                         ("b1", 5), ("b1", 6), ("w", 2),
                         ("b1", 7), ("w", 3)]
                for kind, idx in sched:
                    if kind == "b1":
                        emit_b1(idx)
                    else:
                        for ql in waves[idx]:
                            emit_b2(ql)
                rs_sb = rsp.tile([128, NT], FP32)
                nc.vector.tensor_copy(rs_sb[:], r_ps[:])
                nc.scalar.dma_start(rs_d, rs_sb[:])

        rps_cm.__exit__(None, None, None)
        sps_cm.__exit__(None, None, None)

    nc.compile()
    return nc


def _make_runner(nc):
    """Cached jitted 8-core runner (no donation; avoids per-call re-jit)."""
    import jax
    import numpy as np_
    from jax.sharding import Mesh, PartitionSpec
    from jax.experimental.shard_map import shard_map

    from concourse import mybir
    from concourse.bass2jax import (
        _bass_exec_p,
        install_neuronx_cc_hook,
        partition_id_tensor,
    )

    install_neuronx_cc_hook()
    partition_name = nc.partition_id_tensor.name if nc.partition_id_tensor else None
    in_names, out_names, out_avals = [], [], []
    for alloc in nc.m.functions[0].allocations:
        if not isinstance(alloc, mybir.MemoryLocationSet):
            continue
        name = alloc.memorylocations[0].name
        if alloc.kind == "ExternalInput":
            if name != partition_name:
                in_names.append(name)
        elif alloc.kind == "ExternalOutput":
            out_names.append(name)
            out_avals.append(
                jax.core.ShapedArray(
                    tuple(alloc.tensor_shape), mybir.dt.np(alloc.dtype)
                )
            )
    n_params = len(in_names)
    all_in = list(in_names) + list(out_names)
    if partition_name is not None:
        all_in.append(partition_name)

    def _body(*args):
        operands = list(args)
        if partition_name is not None:
            operands.append(partition_id_tensor())
        return tuple(
            _bass_exec_p.bind(
                *operands,
                out_avals=tuple(out_avals),
                in_names=tuple(all_in),
                out_names=tuple(out_names),
                lowering_input_output_aliases=(),
                sim_require_finite=True,
                sim_require_nnan=True,
                nc=nc,
            )
        )

    devices = jax.devices()[:NCORES]
    mesh = Mesh(np_.asarray(devices), ("core",))
    spec = PartitionSpec("core")
    fn = jax.jit(
        shard_map(
            _body,
            mesh=mesh,
            in_specs=(spec,) * (n_params + len(out_names)),
            out_specs=(spec,) * len(out_names),
            check_rep=False,
        ),
        keep_unused=True,
    )

    def run(in_maps):
        concat_in = [
            np_.concatenate([np_.asarray(m[nm]) for m in in_maps], axis=0)
            for nm in in_names
        ]
        zeros = [
            np_.zeros((NCORES * a.shape[0], *a.shape[1:]), a.dtype) for a in out_avals
        ]
        outs = fn(*concat_in, *zeros)
        return [
            {
                nm: np_.asarray(outs[i]).reshape(NCORES, *out_avals[i].shape)[c]
                for i, nm in enumerate(out_names)
            }
            for c in range(NCORES)
        ]

    return run


def _perm(p):
    return [2 * j + p for j in range(8)] + [2 * i + (1 - p) for i in range(8)]


def _split_fp8(x):
    import ml_dtypes

    f8 = ml_dtypes.float8_e4m3
    h = x.astype(f8)
    l = (x - h.astype(np.float32)).astype(f8)
    return h, l


def _pack_dlayout(t):
    """[1024 d, N] fp32 -> hi/lo fp8 [128 dpart, 4 cc, 2 di, N]."""
    n = t.shape[1]
    r = np.ascontiguousarray(
        t.reshape(4, 2, 128, n).transpose(2, 0, 1, 3)
    )
    return _split_fp8(r)


def prepare_inputs(embeddings, W_Q, W_K, W_V):
    import ml_dtypes

    emb = np.asarray(embeddings, dtype=np.float32)
    ws = {}
    for name, w in (("wq", W_Q), ("wk", W_K), ("wv", W_V)):
        wh, wl = _pack_dlayout(np.asarray(w, dtype=np.float32) * np.float32(WSC))
        ws[name + "h"], ws[name + "l"] = wh, wl

    tri = (np.arange(128)[None, :] >= np.arange(128)[:, None]).astype(np.float32)
    masks = []
    for p in range(2):
        m = np.zeros((128, 256), dtype=np.float32)
        m[:, 0:128] = tri                            # diag: 1 iff q >= k
        m[:, 128:256] = 1.0 if p == 0 else 0.0       # phantom tile mask
        masks.append(m)
    ones_np = np.ones((128, 2, 1), dtype=ml_dtypes.float8_e4m3)

    in_maps = []
    for core in range(NCORES):
        b, p = divmod(core, 2)
        x_t = emb[b].T  # [D, S]
        cols = np.concatenate([np.arange(g * 128, (g + 1) * 128) for g in _perm(p)])
        xp = np.ascontiguousarray(x_t[:, cols])
        xh, xl = _pack_dlayout(xp)
        in_maps.append(
            {
                "xh": xh, "xl": xl,
                "wqh": ws["wqh"], "wql": ws["wql"],
                "wkh": ws["wkh"], "wkl": ws["wkl"],
                "wvh": ws["wvh"], "wvl": ws["wvl"],
                "maskt": masks[p],
                "ones": ones_np,
            }
        )
    return in_maps


def merge_outputs(results):
    out = np.empty((B, S, D), dtype=np.float32)
    for b in range(B):
        osum = np.zeros((S, D), dtype=np.float32)
        rsum = np.zeros((S,), dtype=np.float32)
        for p in range(2):
            r = results[2 * b + p]
            ob = np.asarray(r["ob"]).astype(np.float32)   # [16,128,1024]
            rs = np.asarray(r["rs"]).astype(np.float32)   # [128,16]
            perm = _perm(p)
            for ql in range(NT):
                gq = perm[ql]
                osum[gq * 128:(gq + 1) * 128] += ob[ql]
                rsum[gq * 128:(gq + 1) * 128] += rs[:, ql]
        out[b] = osum / rsum[:, None]
    return out


def kernel(embeddings, W_Q, W_K, W_V):
    from concourse.bass_utils import run_bass_kernel_spmd

    if "nc" not in _CACHE:
        _CACHE["nc"] = _build()
    nc = _CACHE["nc"]

    in_maps = prepare_inputs(embeddings, W_Q, W_K, W_V)

    results = None
    try:
        if "runner" not in _CACHE:
            _CACHE["runner"] = _make_runner(nc)
        results = _CACHE["runner"](in_maps)
    except Exception:
        _CACHE.pop("runner", None)
    if results is None:
        import time as _time

        for attempt in range(2):
            try:
                results = run_bass_kernel_spmd(
                    nc, in_maps, core_ids=list(range(NCORES))
                ).results
                break
            except Exception:
                if attempt == 1:
                    raise
                _time.sleep(3.0)

    return merge_outputs(results)


# revision 5
# speedup vs baseline: 1.1322x; 1.0004x over previous
"""Causal single-head attention (B=4, S=2048, D=1024) on 8 Trainium2 cores.

Sharding: 2 cores per batch, interleaved KEY tiles (core parity p owns global
key tiles 2j+p). Each core computes K^T,V for its 1024 keys, Q^T for all 2048
queries, then S^T-major flash attention over its keys, producing UNNORMALIZED
partial outputs + partial rowsums. The host merges the two cores of a batch:
out = (Ohat_e + Ohat_o) / (rs_e + rs_o).  (No max-subtraction, like the
fp32r baseline; exp arguments are small enough.)

Numerics / speed:
  - Projections run as fp8(e4m3) DoubleRow matmuls with error compensation:
    X^T and 64*W are split on the HOST into hi + lo e4m3 parts and the three
    products hh, hl, lh are accumulated in fp32 PSUM (the tiny lo*lo term is
    dropped). DoubleRow contracts 256 elements per 0.5 cyc/row -> 3x fp32r
    throughput for ~1e-3 relative error. PSUM->SBUF copies rescale by 1/64.
  - Attention (S^T and PV) runs in bf16 (1 cyc/row, same as fp32r, but
    transpose-free): S^T = K^T.T Q^T is computed key-major so exp() writes
    P^T directly in the layout PV needs; rowsums come from an extra ap=1
    matmul against a ones vector reusing the PV stationary.
  - The program is SPMD-identical on all 8 cores; parity enters only through
    the host-packed column permutation of X^T and two mask tiles (diagonal
    tri mask + a phantom-tile mask that zeroes the odd core's extra tile).
"""

import numpy as np

B, S, D = 4, 2048, 1024
NCORES = 8
NT = 16             # 128-row tiles per sequence
SCALE = 1.0 / np.sqrt(np.float32(D))
# weight pre-scale for fp8 (W ~ +-0.054 is subnormal in e4m3). K^T/Q^T stay in
# x32-scaled form in SBUF (so hi/lo splitting is a plain copy + subtract and
# 32*|K| < 240 stays in e4m3 range); the 1/(32*32) comes out in the exp scale.
# V is unscaled during its PSUM copy.
WSC = 32.0

_CACHE = {}


def _build(cfg=None):
    from contextlib import ExitStack

    from concourse import bacc
    import concourse.mybir as mybir
    import concourse.tile as tile

    cfg = cfg or {}
    FP32 = mybir.dt.float32
    F8 = mybir.dt.float8e4
    BF16 = mybir.dt.bfloat16
    DR = mybir.MatmulPerfMode.DoubleRow
    EXP = mybir.ActivationFunctionType.Exp
    COPY = mybir.ActivationFunctionType.Copy
    ADD = mybir.AluOpType.add
    MULT = mybir.AluOpType.mult

    nc = bacc.Bacc("TRN2", debug=False, num_devices=NCORES, dynamic_dma_scratch_size=4096)

    # X^T in permuted column order (core's own key tiles first), fp8 hi/lo.
    # layout [dpart, cc, di, col]: contraction index d = cc*256 + di*128 + dpart
    xh_d = nc.dram_tensor("xh", [128, 4, 2, 2048], F8, kind="ExternalInput").ap()
    xl_d = nc.dram_tensor("xl", [128, 4, 2, 2048], F8, kind="ExternalInput").ap()
    w_d = {}
    for wn in ("wk", "wq", "wv"):
        for part in ("h", "l"):
            w_d[wn + part] = nc.dram_tensor(
                wn + part, [128, 4, 2, 1024], F8, kind="ExternalInput"
            ).ap()
    # masks: [:,0:128] diag tri (0 if q>=k else -1e9); [:,128:256] phantom
    # (all 0 on even cores, all -1e9 on odd cores)
    mask_d = nc.dram_tensor("maskt", [128, 256], FP32, kind="ExternalInput").ap()
    ones_d = nc.dram_tensor("ones", [128, 2, 1], F8, kind="ExternalInput").ap()
    ob_d = nc.dram_tensor("ob", [NT, 128, D], BF16, kind="ExternalOutput").ap()
    rs_d = nc.dram_tensor("rs", [128, NT], FP32, kind="ExternalOutput").ap()

    INV = float(1.0 / WSC)

    with tile.TileContext(nc) as tc, ExitStack() as ctx:
        const = ctx.enter_context(tc.tile_pool(name="const", bufs=1))
        resident = ctx.enter_context(tc.tile_pool(name="resident", bufs=1))

        mask_sb = const.tile([128, 256], FP32)
        ones_sb = const.tile([128, 2, 1], F8)
        nln4_sb = const.tile([128, 1], FP32)
        nc.gpsimd.memset(nln4_sb[:], -1.3862943611198906)
        # ramp warm-up: idle advances the p-state clock for free, so fill the
        # ~5.8us DMA launch window with dummy matmuls; real work then starts
        # at the full 2.4GHz instead of paying ~3us of half-clock cycles
        warm_sb = const.tile([128, 512], BF16)
        nc.gpsimd.memset(warm_sb[:], 0.0)

        # K^T/Q^T: x32-scaled fp8 hi/lo in DoubleRow layout [dpart, cc, di, col]
        kth = resident.tile([128, 4, 2, 1024], F8)
        ktl = resident.tile([128, 4, 2, 1024], F8)
        qth = resident.tile([128, 4, 2, 2048], F8)
        qtl = resident.tile([128, 4, 2, 2048], F8)
        # V: x32-scaled fp8 hi/lo in pair layout [kpart, pair a, sub, dv]
        vh = resident.tile([128, 4, 2, 1024], F8)
        vl = resident.tile([128, 4, 2, 1024], F8)

        # ---------------- Phase A: projections (fp8 DR, 3-term) ----------------
        # NOTE: GPSIMD/Pool cannot access PSUM on real HW -> DVE/Act only
        SUB = mybir.AluOpType.subtract

        def psum_split_f8(dsth, dstl, src):
            """hi = f8(psum); lo = f8(psum - hi). Keeps the x32 scale."""
            nc.scalar.activation(dsth, src, COPY)
            nc.vector.tensor_tensor(dstl, src, dsth, SUB)

        def psum_copy_scaled(i, dst, src):
            if i % 2 == 0:
                nc.scalar.activation(dst, src, COPY, scale=INV)
            else:
                nc.vector.tensor_scalar_mul(dst, src, INV)

        # PSUM plan (8 banks): apsum(4) for projections; spsum(2)+rpsum(1)
        # co-resident so B1 needs no pool barrier; opsum(2x2) replaces apsum
        # during B1 so B1->B2 needs no barrier either. Pools close LIFO, so
        # the long-lived spsum/rpsum open first.
        sps_cm = tc.tile_pool(name="spsum", bufs=cfg.get("sbufs", 3), space="PSUM")
        sps = sps_cm.__enter__()
        rps_cm = tc.tile_pool(name="rpsum", bufs=1, space="PSUM")
        rps = rps_cm.__enter__()
        aps_cm = tc.tile_pool(name="apsum", bufs=cfg.get("abufs", 4), space="PSUM")
        aps = aps_cm.__enter__()

        wps = aps.tile([128, 512], FP32, tag="ps", name="warm")
        for wi in range(cfg.get("warmups", 4)):
            nc.tensor.matmul(
                wps[:], warm_sb[:, 0:128], warm_sb[:],
                start=True, stop=True, skip_group_check=True,
            )

        with tc.tile_pool(name="xpool", bufs=1) as xp, \
             tc.tile_pool(name="wpool", bufs=6) as wp:

            xh = xp.tile([128, 4, 2, 2048], F8)
            xl = xp.tile([128, 4, 2, 2048], F8)
            # whole 512-col chunks: per-cc mini-splitting is HWDGE-trigger
            # bound (625ns/DMA vs 364ns transfer) and starves the lo chunks
            nc.sync.dma_start(xh[:, :, :, 0:512], xh_d[:, :, :, 0:512])

            def load_w(name):
                h = wp.tile([128, 4, 2, 1024], F8, name=name + "h", tag="w")
                l = wp.tile([128, 4, 2, 1024], F8, name=name + "l", tag="w")
                q = nc.scalar
                if name == "wk":
                    for m0, m1 in ((0, 512), (512, 1024)):
                        q.dma_start(h[:, :, :, m0:m1], w_d[name + "h"][:, :, :, m0:m1])
                        q.dma_start(l[:, :, :, m0:m1], w_d[name + "l"][:, :, :, m0:m1])
                else:
                    q.dma_start(h[:], w_d[name + "h"])
                    q.dma_start(l[:], w_d[name + "l"])
                return h, l

            def proj_tile(ci, ps, wh, wl, xcols, m):
                """ps[128,512] += sum_cc (W^T X)[m-chunk, xcols] via 12 DR matmuls.

                hh terms first: the first tile can start before the lo
                tensors have even arrived from HBM."""
                terms = []
                for cc in range(4):
                    terms.append((wh[:, cc, :, m * 128:(m + 1) * 128], xh[:, cc, :, xcols]))
                for cc in range(4):
                    terms.append((wh[:, cc, :, m * 128:(m + 1) * 128], xl[:, cc, :, xcols]))
                for cc in range(4):
                    terms.append((wl[:, cc, :, m * 128:(m + 1) * 128], xh[:, cc, :, xcols]))
                for i, (lt, rt) in enumerate(terms):
                    nc.tensor.matmul(
                        ps[:], lt, rt,
                        start=(i == 0), stop=(i == len(terms) - 1),
                        perf_mode=DR,
                    )

            def vproj_tile(ps, xcols, wvh, wvl, h):
                terms = []
                for cc in range(4):
                    terms.append((xh[:, cc, :, xcols], wvh[:, cc, :, h * 512:(h + 1) * 512]))
                for cc in range(4):
                    terms.append((xh[:, cc, :, xcols], wvl[:, cc, :, h * 512:(h + 1) * 512]))
                for cc in range(4):
                    terms.append((xl[:, cc, :, xcols], wvh[:, cc, :, h * 512:(h + 1) * 512]))
                for i, (lt, rt) in enumerate(terms):
                    nc.tensor.matmul(
                        ps[:], lt, rt,
                        start=(i == 0), stop=(i == len(terms) - 1),
                        perf_mode=DR,
                    )

            ci = 0
            # K^T: own keys = first 1024 permuted columns
            wkh, wkl = load_w("wk")
            # remaining X chunks split across BOTH queues so the W streams
            # don't starve K-proj of its x-chunks on the shared DMA engines
            nc.sync.dma_start(xl[:, :, :, 0:512], xl_d[:, :, :, 0:512])
            nc.sync.dma_start(xh[:, :, :, 512:1024], xh_d[:, :, :, 512:1024])
            nc.scalar.dma_start(xl[:, :, :, 512:1024], xl_d[:, :, :, 512:1024])
            nc.sync.dma_start(xh[:, :, :, 1024:1536], xh_d[:, :, :, 1024:1536])
            nc.scalar.dma_start(xl[:, :, :, 1024:1536], xl_d[:, :, :, 1024:1536])
            nc.sync.dma_start(xh[:, :, :, 1536:2048], xh_d[:, :, :, 1536:2048])
            nc.sync.dma_start(xl[:, :, :, 1536:2048], xl_d[:, :, :, 1536:2048])
            for kc in range(2):
                for m in range(8):
                    ps = aps.tile([128, 512], FP32, tag="ps")
                    proj_tile(ci, ps, wkh, wkl, slice(kc * 512, (kc + 1) * 512), m)
                    ks = (slice(None), m // 2, m % 2, slice(kc * 512, (kc + 1) * 512))
                    psum_split_f8(kth[ks], ktl[ks], ps[:])
                    ci += 1
            # Q^T: all 2048 columns
            wqh, wql = load_w("wq")
            for qc in range(4):
                for m in range(8):
                    ps = aps.tile([128, 512], FP32, tag="ps")
                    proj_tile(ci, ps, wqh, wql, slice(qc * 512, (qc + 1) * 512), m)
                    qs = (slice(None), m // 2, m % 2, slice(qc * 512, (qc + 1) * 512))
                    psum_split_f8(qth[qs], qtl[qs], ps[:])
                    ci += 1
            # V: own key tiles as stationary, W_V as moving
            wvh, wvl = load_w("wv")
            nc.scalar.dma_start(mask_sb[:], mask_d)
            nc.scalar.dma_start(ones_sb[:], ones_d)
            for j in range(8):
                for h in range(2):
                    ps = aps.tile([128, 512], FP32, tag="ps")
                    vproj_tile(ps, slice(j * 128, (j + 1) * 128), wvh, wvl, h)
                    vs = (slice(None), j // 2, j % 2, slice(h * 512, (h + 1) * 512))
                    psum_split_f8(vh[vs], vl[vs], ps[:])
                    ci += 1

        # ---------------- Phase B1: S^T + exp -> P^T (bf16) ----------------
        # P^T[j]: [128 keys, own (8-j)*128 | other (8-j)*128] columns
        aps_cm.__exit__(None, None, None)

        # P^T pair tiles (fp8 hi/lo): pair a covers kts j=2a (sub 0) and
        # j=2a+1 (sub 1, shifted one qtile; its two leading 128-col regions
        # are zeroed so DoubleRow PV over the pair is uniformly correct).
        LN4 = 1.3862943611198906
        pth, ptl = [], []
        with tc.tile_pool(name="ptpool", bufs=1) as ptp, \
             tc.tile_pool(name="ptmp", bufs=6) as ptmpp:
            for a in range(4):
                nqa = 2 * (8 - 2 * a) * 128
                pth.append(ptp.tile([128, 2, nqa], F8, name=f"pth{a}"))
                ptl.append(ptp.tile([128, 2, nqa], F8, name=f"ptl{a}"))
            for a in range(4):
                La = (8 - 2 * a) * 128
                for t in (pth[a], ptl[a]):
                    nc.gpsimd.memset(t[:, 1, 0:128], 0.0)
                    nc.gpsimd.memset(t[:, 1, La:La + 128], 0.0)

            # B2 pools co-resident with B1's: spsum(3)+rpsum(1)+opsum(4) = 8
            with tc.tile_pool(name="opsum", bufs=cfg.get("obufs", 4), space="PSUM") as ops, \
                 tc.tile_pool(name="ostage", bufs=cfg.get("ostb", 6)) as osp, \
                 tc.tile_pool(name="rstage", bufs=1) as rsp:  # noqa: F841

                r_ps = rps.tile([128, NT], FP32)

                def emit_b1(j):
                    a, sub = j // 2, j % 2
                    La = (8 - 2 * a) * 128
                    seg = (8 - j) * 128
                    for half in range(2):      # 0 = own, 1 = other
                        q0 = j * 128 + half * 1024
                        off = half * La + sub * 128
                        done = 0
                        while done < seg:
                            cw = min(512, seg - done)
                            ps = sps.tile([128, 512], FP32, tag="s")
                            kcols = slice(j * 128, (j + 1) * 128)
                            qcols = slice(q0 + done, q0 + done + cw)
                            terms = []
                            for cc in range(4):
                                terms.append((kth[:, cc, :, kcols], qth[:, cc, :, qcols]))
                            for cc in range(4):
                                terms.append((kth[:, cc, :, kcols], qtl[:, cc, :, qcols]))
                            for cc in range(4):
                                terms.append((ktl[:, cc, :, kcols], qth[:, cc, :, qcols]))
                            for i, (lt, rt) in enumerate(terms):
                                nc.tensor.matmul(
                                    ps[:, 0:cw], lt, rt,
                                    start=(i == 0), stop=(i == len(terms) - 1),
                                    perf_mode=DR,
                                )
                            # 2-pass: exp -> fp32 tmp (Act); 0/1 mask on the
                            # SBUF tmp (DVE, off the PSUM critical path);
                            # hi on Pool, lo on DVE
                            ptmp = ptmpp.tile([128, 512], FP32, tag="pt")
                            nc.scalar.activation(
                                ptmp[:, 0:cw], ps[:, 0:cw], EXP,
                                scale=float(SCALE / (WSC * WSC)), bias=nln4_sb[:],
                            )
                            if done == 0:
                                nc.vector.tensor_tensor(
                                    ptmp[:, 0:128], ptmp[:, 0:128],
                                    mask_sb[:, half * 128:half * 128 + 128], MULT,
                                )
                            dsth = pth[a][:, sub, off + done:off + done + cw]
                            dstl = ptl[a][:, sub, off + done:off + done + cw]
                            nc.gpsimd.tensor_copy(dsth, ptmp[:, 0:cw])
                            nc.vector.tensor_tensor(dstl, ptmp[:, 0:cw], dsth, SUB)
                            done += cw

                def pv_col(ql, a):
                    La = (8 - 2 * a) * 128
                    if ql < 8:
                        return (ql - 2 * a) * 128
                    return La + (ql - 8 - 2 * a) * 128

                def emit_b2(ql, tail=False):
                    # own half: ql = j' -> jlim = j'; other: ql = 8+i -> jlim = i
                    jlim = ql if ql < 8 else ql - 8
                    alim = jlim // 2
                    o_sb = osp.tile([128, D], BF16, tag="ob")
                    for a in range(alim + 1):
                        c = pv_col(ql, a)
                        for i, pt_ in enumerate((pth[a], ptl[a])):
                            nc.tensor.matmul(
                                r_ps[:, ql:ql + 1], pt_[:, :, c:c + 128], ones_sb[:],
                                start=(a == 0 and i == 0), stop=(a == alim and i == 1),
                                perf_mode=DR, skip_group_check=True,
                            )
                    for h in range(2):
                        o_ps = ops.tile([128, 512], FP32, tag="o")
                        hs = slice(h * 512, (h + 1) * 512)
                        nterm = 3 * (alim + 1)
                        i = 0
                        for a in range(alim + 1):
                            c = pv_col(ql, a)
                            for pt_, v_ in ((pth[a], vh), (pth[a], vl), (ptl[a], vh)):
                                nc.tensor.matmul(
                                    o_ps[:],
                                    pt_[:, :, c:c + 128], v_[:, a, :, hs],
                                    start=(i == 0), stop=(i == nterm - 1),
                                    perf_mode=DR, skip_group_check=True,
                                )
                                i += 1
                        # copy each 512-half as soon as its group stops (and
                        # undo the x32 V scale); the other half's matmuls hide
                        # the copy+DMA latency
                        if tail and h == 1:
                            # final tile: quarter-split so copy/DMA pipeline
                            # and the exposed tail is one quarter, not a half
                            for q4 in range(4):
                                qs = slice(q4 * 128, (q4 + 1) * 128)
                                gs = slice(h * 512 + q4 * 128, h * 512 + (q4 + 1) * 128)
                                if q4 % 2 == 0:
                                    nc.scalar.activation(o_sb[:, gs], o_ps[:, qs], COPY, scale=INV)
                                else:
                                    nc.vector.tensor_scalar_mul(o_sb[:, gs], o_ps[:, qs], INV)
                                nc.sync.dma_start(ob_d[ql][:, gs], o_sb[:, gs])
                        elif (ql + h) % 2 == 0:
                            nc.scalar.activation(o_sb[:, hs], o_ps[:], COPY, scale=INV)
                            nc.sync.dma_start(ob_d[ql][:, hs], o_sb[:, hs])
                        else:
                            nc.vector.tensor_scalar_mul(o_sb[:, hs], o_ps[:], INV)
                            nc.sync.dma_start(ob_d[ql][:, hs], o_sb[:, hs])

                # interleave: each P^T pair a finalizes after B1 j=2a,2a+1, at
                # which point the B2 qtiles needing only pairs <= a can run.
                # Kills the B1->B2 phase boundary and spreads the copy load.
                # each wave's B2 runs one B1 iteration late so the exp->
                # hi/lo chains of its pair are fully drained when it starts
                waves = [[0, 1, 8, 9], [2, 3, 10, 11], [4, 5, 12, 13], [6, 7, 15, 14]]
                sched = [("b1", 0), ("b1", 1), ("b1", 2), ("w", 0),
                         ("b1", 3), ("b1", 4), ("w", 1),
                         ("b1", 5), ("b1", 6), ("w", 2),
                         ("b1", 7), ("w", 3)]
                for kind, idx in sched:
                    if kind == "b1":
                        emit_b1(idx)
                    else:
                        for ql in waves[idx]:
                            emit_b2(ql)
                rs_sb = rsp.tile([128, NT], FP32)
                nc.vector.tensor_copy(rs_sb[:], r_ps[:])
                nc.scalar.dma_start(rs_d, rs_sb[:])

        rps_cm.__exit__(None, None, None)
        sps_cm.__exit__(None, None, None)

    nc.compile()
    return nc


def _make_runner(nc):
    """Cached jitted 8-core runner (no donation; avoids per-call re-jit)."""
    import jax
    import numpy as np_
    from jax.sharding import Mesh, PartitionSpec
    from jax.experimental.shard_map import shard_map

    from concourse import mybir
    from concourse.bass2jax import (
        _bass_exec_p,
        install_neuronx_cc_hook,
        partition_id_tensor,
    )

    install_neuronx_cc_hook()
    partition_name = nc.partition_id_tensor.name if nc.partition_id_tensor else None
    in_names, out_names, out_avals = [], [], []
    for alloc in nc.m.functions[0].allocations:
        if not isinstance(alloc, mybir.MemoryLocationSet):
            continue
        name = alloc.memorylocations[0].name
        if alloc.kind == "ExternalInput":
            if name != partition_name:
                in_names.append(name)
        elif alloc.kind == "ExternalOutput":
            out_names.append(name)
            out_avals.append(
                jax.core.ShapedArray(
                    tuple(alloc.tensor_shape), mybir.dt.np(alloc.dtype)
                )
            )
    n_params = len(in_names)
    all_in = list(in_names) + list(out_names)
    if partition_name is not None:
        all_in.append(partition_name)

    def _body(*args):
        operands = list(args)
        if partition_name is not None:
            operands.append(partition_id_tensor())
        return tuple(
            _bass_exec_p.bind(
                *operands,
                out_avals=tuple(out_avals),
                in_names=tuple(all_in),
                out_names=tuple(out_names),
                lowering_input_output_aliases=(),
                sim_require_finite=True,
                sim_require_nnan=True,
                nc=nc,
            )
        )

    devices = jax.devices()[:NCORES]
    mesh = Mesh(np_.asarray(devices), ("core",))
    spec = PartitionSpec("core")
    fn = jax.jit(
        shard_map(
            _body,
            mesh=mesh,
            in_specs=(spec,) * (n_params + len(out_names)),
            out_specs=(spec,) * len(out_names),
            check_rep=False,
        ),
        keep_unused=True,
    )

    def run(in_maps):
        concat_in = [
            np_.concatenate([np_.asarray(m[nm]) for m in in_maps], axis=0)
            for nm in in_names
        ]
        zeros = [
            np_.zeros((NCORES * a.shape[0], *a.shape[1:]), a.dtype) for a in out_avals
        ]
        outs = fn(*concat_in, *zeros)
        return [
            {
                nm: np_.asarray(outs[i]).reshape(NCORES, *out_avals[i].shape)[c]
                for i, nm in enumerate(out_names)
            }
            for c in range(NCORES)
        ]

    return run


def _perm(p):
    return [2 * j + p for j in range(8)] + [2 * i + (1 - p) for i in range(8)]


def _split_fp8(x):
    import ml_dtypes

    f8 = ml_dtypes.float8_e4m3
    h = x.astype(f8)
    l = (x - h.astype(np.float32)).astype(f8)
    return h, l


def _pack_dlayout(t):
    """[1024 d, N] fp32 -> hi/lo fp8 [128 dpart, 4 cc, 2 di, N]."""
    n = t.shape[1]
    r = np.ascontiguousarray(
        t.reshape(4, 2, 128, n).transpose(2, 0, 1, 3)
    )
    return _split_fp8(r)


def prepare_inputs(embeddings, W_Q, W_K, W_V):
    import ml_dtypes

    emb = np.asarray(embeddings, dtype=np.float32)
    ws = {}
    for name, w in (("wq", W_Q), ("wk", W_K), ("wv", W_V)):
        wh, wl = _pack_dlayout(np.asarray(w, dtype=np.float32) * np.float32(WSC))
        ws[name + "h"], ws[name + "l"] = wh, wl

    tri = (np.arange(128)[None, :] >= np.arange(128)[:, None]).astype(np.float32)
    masks = []
    for p in range(2):
        m = np.zeros((128, 256), dtype=np.float32)
        m[:, 0:128] = tri                            # diag: 1 iff q >= k
        m[:, 128:256] = 1.0 if p == 0 else 0.0       # phantom tile mask
        masks.append(m)
    ones_np = np.ones((128, 2, 1), dtype=ml_dtypes.float8_e4m3)

    in_maps = []
    for core in range(NCORES):
        b, p = divmod(core, 2)
        x_t = emb[b].T  # [D, S]
        cols = np.concatenate([np.arange(g * 128, (g + 1) * 128) for g in _perm(p)])
        xp = np.ascontiguousarray(x_t[:, cols])
        xh, xl = _pack_dlayout(xp)
        in_maps.append(
            {
                "xh": xh, "xl": xl,
                "wqh": ws["wqh"], "wql": ws["wql"],
                "wkh": ws["wkh"], "wkl": ws["wkl"],
                "wvh": ws["wvh"], "wvl": ws["wvl"],
                "maskt": masks[p],
                "ones": ones_np,
            }
        )
    return in_maps


def merge_outputs(results):
    out = np.empty((B, S, D), dtype=np.float32)
    for b in range(B):
        osum = np.zeros((S, D), dtype=np.float32)
        rsum = np.zeros((S,), dtype=np.float32)
        for p in range(2):
            r = results[2 * b + p]
            ob = np.asarray(r["ob"]).astype(np.float32)   # [16,128,1024]
            rs = np.asarray(r["rs"]).astype(np.float32)   # [128,16]
            perm = _perm(p)
            for ql in range(NT):
                gq = perm[ql]
                osum[gq * 128:(gq + 1) * 128] += ob[ql]
                rsum[gq * 128:(gq + 1) * 128] += rs[:, ql]
        out[b] = osum / rsum[:, None]
    return out


def kernel(embeddings, W_Q, W_K, W_V):
    from concourse.bass_utils import run_bass_kernel_spmd

    if "nc" not in _CACHE:
        _CACHE["nc"] = _build()
    nc = _CACHE["nc"]

    in_maps = prepare_inputs(embeddings, W_Q, W_K, W_V)

    results = None
    try:
        if "runner" not in _CACHE:
            _CACHE["runner"] = _make_runner(nc)
        results = _CACHE["runner"](in_maps)
    except Exception:
        _CACHE.pop("runner", None)
    if results is None:
        import time as _time

        for attempt in range(2):
            try:
                results = run_bass_kernel_spmd(
                    nc, in_maps, core_ids=list(range(NCORES))
                ).results
                break
            except Exception:
                if attempt == 1:
                    raise
                _time.sleep(3.0)

    return merge_outputs(results)


# revision 6
# speedup vs baseline: 1.1322x; 1.0000x over previous
"""Causal single-head attention (B=4, S=2048, D=1024) on 8 Trainium2 cores.

Sharding: 2 cores per batch, interleaved KEY tiles (core parity p owns global
key tiles 2j+p). Each core computes K^T,V for its 1024 keys, Q^T for all 2048
queries, then S^T-major flash attention over its keys, producing UNNORMALIZED
partial outputs + partial rowsums. The host merges the two cores of a batch:
out = (Ohat_e + Ohat_o) / (rs_e + rs_o).  (No max-subtraction, like the
fp32r baseline; exp arguments are small enough.)

Numerics / speed:
  - Projections run as fp8(e4m3) DoubleRow matmuls with error compensation:
    X^T and 64*W are split on the HOST into hi + lo e4m3 parts and the three
    products hh, hl, lh are accumulated in fp32 PSUM (the tiny lo*lo term is
    dropped). DoubleRow contracts 256 elements per 0.5 cyc/row -> 3x fp32r
    throughput for ~1e-3 relative error. PSUM->SBUF copies rescale by 1/64.
  - Attention (S^T and PV) runs in bf16 (1 cyc/row, same as fp32r, but
    transpose-free): S^T = K^T.T Q^T is computed key-major so exp() writes
    P^T directly in the layout PV needs; rowsums come from an extra ap=1
    matmul against a ones vector reusing the PV stationary.
  - The program is SPMD-identical on all 8 cores; parity enters only through
    the host-packed column permutation of X^T and two mask tiles (diagonal
    tri mask + a phantom-tile mask that zeroes the odd core's extra tile).
"""

import numpy as np

B, S, D = 4, 2048, 1024
NCORES = 8
NT = 16             # 128-row tiles per sequence
SCALE = 1.0 / np.sqrt(np.float32(D))
# weight pre-scale for fp8 (W ~ +-0.054 is subnormal in e4m3). K^T/Q^T stay in
# x32-scaled form in SBUF (so hi/lo splitting is a plain copy + subtract and
# 32*|K| < 240 stays in e4m3 range); the 1/(32*32) comes out in the exp scale.
# V is unscaled during its PSUM copy.
WSC = 32.0

_CACHE = {}


def _build(cfg=None):
    from contextlib import ExitStack

    from concourse import bacc
    import concourse.mybir as mybir
    import concourse.tile as tile

    cfg = cfg or {}
    FP32 = mybir.dt.float32
    F8 = mybir.dt.float8e4
    BF16 = mybir.dt.bfloat16
    DR = mybir.MatmulPerfMode.DoubleRow
    EXP = mybir.ActivationFunctionType.Exp
    COPY = mybir.ActivationFunctionType.Copy
    ADD = mybir.AluOpType.add
    MULT = mybir.AluOpType.mult

    nc = bacc.Bacc("TRN2", debug=False, num_devices=NCORES, dynamic_dma_scratch_size=4096)

    # X^T in permuted column order (core's own key tiles first), fp8 hi/lo.
    # layout [dpart, cc, di, col]: contraction index d = cc*256 + di*128 + dpart
    xh_d = nc.dram_tensor("xh", [128, 4, 2, 2048], F8, kind="ExternalInput").ap()
    xl_d = nc.dram_tensor("xl", [128, 4, 2, 2048], F8, kind="ExternalInput").ap()
    w_d = {}
    for wn in ("wk", "wq", "wv"):
        for part in ("h", "l"):
            w_d[wn + part] = nc.dram_tensor(
                wn + part, [128, 4, 2, 1024], F8, kind="ExternalInput"
            ).ap()
    # masks: [:,0:128] diag tri (0 if q>=k else -1e9); [:,128:256] phantom
    # (all 0 on even cores, all -1e9 on odd cores)
    mask_d = nc.dram_tensor("maskt", [128, 256], FP32, kind="ExternalInput").ap()
    ones_d = nc.dram_tensor("ones", [128, 2, 1], F8, kind="ExternalInput").ap()
    ob_d = nc.dram_tensor("ob", [NT, 128, D], BF16, kind="ExternalOutput").ap()
    rs_d = nc.dram_tensor("rs", [128, NT], FP32, kind="ExternalOutput").ap()

    INV = float(1.0 / WSC)

    with tile.TileContext(nc) as tc, ExitStack() as ctx:
        const = ctx.enter_context(tc.tile_pool(name="const", bufs=1))
        resident = ctx.enter_context(tc.tile_pool(name="resident", bufs=1))

        mask_sb = const.tile([128, 256], FP32)
        ones_sb = const.tile([128, 2, 1], F8)
        nln4_sb = const.tile([128, 1], FP32)
        nc.gpsimd.memset(nln4_sb[:], -1.3862943611198906)
        # ramp warm-up: idle advances the p-state clock for free, so fill the
        # ~5.8us DMA launch window with dummy matmuls; real work then starts
        # at the full 2.4GHz instead of paying ~3us of half-clock cycles
        warm_sb = const.tile([128, 512], BF16)
        nc.gpsimd.memset(warm_sb[:], 0.0)

        # K^T/Q^T: x32-scaled fp8 hi/lo in DoubleRow layout [dpart, cc, di, col]
        kth = resident.tile([128, 4, 2, 1024], F8)
        ktl = resident.tile([128, 4, 2, 1024], F8)
        qth = resident.tile([128, 4, 2, 2048], F8)
        qtl = resident.tile([128, 4, 2, 2048], F8)
        # V: x32-scaled fp8 hi/lo in pair layout [kpart, pair a, sub, dv]
        vh = resident.tile([128, 4, 2, 1024], F8)
        vl = resident.tile([128, 4, 2, 1024], F8)

        # ---------------- Phase A: projections (fp8 DR, 3-term) ----------------
        # NOTE: GPSIMD/Pool cannot access PSUM on real HW -> DVE/Act only
        SUB = mybir.AluOpType.subtract

        def psum_split_f8(dsth, dstl, src):
            """hi = f8(psum); lo = f8(psum - hi). Keeps the x32 scale."""
            nc.scalar.activation(dsth, src, COPY)
            nc.vector.tensor_tensor(dstl, src, dsth, SUB)

        def psum_copy_scaled(i, dst, src):
            if i % 2 == 0:
                nc.scalar.activation(dst, src, COPY, scale=INV)
            else:
                nc.vector.tensor_scalar_mul(dst, src, INV)

        # PSUM plan (8 banks): apsum(4) for projections; spsum(2)+rpsum(1)
        # co-resident so B1 needs no pool barrier; opsum(2x2) replaces apsum
        # during B1 so B1->B2 needs no barrier either. Pools close LIFO, so
        # the long-lived spsum/rpsum open first.
        sps_cm = tc.tile_pool(name="spsum", bufs=cfg.get("sbufs", 3), space="PSUM")
        sps = sps_cm.__enter__()
        rps_cm = tc.tile_pool(name="rpsum", bufs=1, space="PSUM")
        rps = rps_cm.__enter__()
        aps_cm = tc.tile_pool(name="apsum", bufs=cfg.get("abufs", 4), space="PSUM")
        aps = aps_cm.__enter__()

        wps = aps.tile([128, 512], FP32, tag="ps", name="warm")
        for wi in range(cfg.get("warmups", 4)):
            nc.tensor.matmul(
                wps[:], warm_sb[:, 0:128], warm_sb[:],
                start=True, stop=True, skip_group_check=True,
            )

        with tc.tile_pool(name="xpool", bufs=1) as xp, \
             tc.tile_pool(name="wpool", bufs=6) as wp:

            xh = xp.tile([128, 4, 2, 2048], F8)
            xl = xp.tile([128, 4, 2, 2048], F8)
            # whole 512-col chunks except the very first: splitting xh0 and
            # wkh-h0 into 728ns cc-halves (still above the 625ns trigger
            # floor, still 512B elems) gates the first hh matmul ~1.5us
            # earlier without slowing the stream
            nc.sync.dma_start(xh[:, 0:2, :, 0:512], xh_d[:, 0:2, :, 0:512])
            nc.sync.dma_start(xh[:, 2:4, :, 0:512], xh_d[:, 2:4, :, 0:512])

            def load_w(name):
                h = wp.tile([128, 4, 2, 1024], F8, name=name + "h", tag="w")
                l = wp.tile([128, 4, 2, 1024], F8, name=name + "l", tag="w")
                q = nc.scalar
                if name == "wk":
                    q.dma_start(h[:, 0:2, :, 0:512], w_d[name + "h"][:, 0:2, :, 0:512])
                    q.dma_start(h[:, 2:4, :, 0:512], w_d[name + "h"][:, 2:4, :, 0:512])
                    q.dma_start(l[:, :, :, 0:512], w_d[name + "l"][:, :, :, 0:512])
                    q.dma_start(h[:, :, :, 512:1024], w_d[name + "h"][:, :, :, 512:1024])
                    q.dma_start(l[:, :, :, 512:1024], w_d[name + "l"][:, :, :, 512:1024])
                else:
                    q.dma_start(h[:], w_d[name + "h"])
                    q.dma_start(l[:], w_d[name + "l"])
                return h, l

            def proj_tile(ci, ps, wh, wl, xcols, m):
                """ps[128,512] += sum_cc (W^T X)[m-chunk, xcols] via 12 DR matmuls.

                hh terms first: the first tile can start before the lo
                tensors have even arrived from HBM."""
                terms = []
                for cc in range(4):
                    terms.append((wh[:, cc, :, m * 128:(m + 1) * 128], xh[:, cc, :, xcols]))
                for cc in range(4):
                    terms.append((wh[:, cc, :, m * 128:(m + 1) * 128], xl[:, cc, :, xcols]))
                for cc in range(4):
                    terms.append((wl[:, cc, :, m * 128:(m + 1) * 128], xh[:, cc, :, xcols]))
                for i, (lt, rt) in enumerate(terms):
                    nc.tensor.matmul(
                        ps[:], lt, rt,
                        start=(i == 0), stop=(i == len(terms) - 1),
                        perf_mode=DR,
                    )

            def vproj_tile(ps, xcols, wvh, wvl, h):
                terms = []
                for cc in range(4):
                    terms.append((xh[:, cc, :, xcols], wvh[:, cc, :, h * 512:(h + 1) * 512]))
                for cc in range(4):
                    terms.append((xh[:, cc, :, xcols], wvl[:, cc, :, h * 512:(h + 1) * 512]))
                for cc in range(4):
                    terms.append((xl[:, cc, :, xcols], wvh[:, cc, :, h * 512:(h + 1) * 512]))
                for i, (lt, rt) in enumerate(terms):
                    nc.tensor.matmul(
                        ps[:], lt, rt,
                        start=(i == 0), stop=(i == len(terms) - 1),
                        perf_mode=DR,
                    )

            ci = 0
            # K^T: own keys = first 1024 permuted columns
            wkh, wkl = load_w("wk")
            # remaining X chunks split across BOTH queues so the W streams
            # don't starve K-proj of its x-chunks on the shared DMA engines
            nc.sync.dma_start(xl[:, :, :, 0:512], xl_d[:, :, :, 0:512])
            nc.sync.dma_start(xh[:, :, :, 512:1024], xh_d[:, :, :, 512:1024])
            nc.scalar.dma_start(xl[:, :, :, 512:1024], xl_d[:, :, :, 512:1024])
            nc.sync.dma_start(xh[:, :, :, 1024:1536], xh_d[:, :, :, 1024:1536])
            nc.scalar.dma_start(xl[:, :, :, 1024:1536], xl_d[:, :, :, 1024:1536])
            nc.sync.dma_start(xh[:, :, :, 1536:2048], xh_d[:, :, :, 1536:2048])
            nc.sync.dma_start(xl[:, :, :, 1536:2048], xl_d[:, :, :, 1536:2048])
            for kc in range(2):
                for m in range(8):
                    ps = aps.tile([128, 512], FP32, tag="ps")
                    proj_tile(ci, ps, wkh, wkl, slice(kc * 512, (kc + 1) * 512), m)
                    ks = (slice(None), m // 2, m % 2, slice(kc * 512, (kc + 1) * 512))
                    psum_split_f8(kth[ks], ktl[ks], ps[:])
                    ci += 1
            # Q^T: all 2048 columns
            wqh, wql = load_w("wq")
            for qc in range(4):
                for m in range(8):
                    ps = aps.tile([128, 512], FP32, tag="ps")
                    proj_tile(ci, ps, wqh, wql, slice(qc * 512, (qc + 1) * 512), m)
                    qs = (slice(None), m // 2, m % 2, slice(qc * 512, (qc + 1) * 512))
                    psum_split_f8(qth[qs], qtl[qs], ps[:])
                    ci += 1
            # V: own key tiles as stationary, W_V as moving
            wvh, wvl = load_w("wv")
            nc.scalar.dma_start(mask_sb[:], mask_d)
            nc.scalar.dma_start(ones_sb[:], ones_d)
            for j in range(8):
                for h in range(2):
                    ps = aps.tile([128, 512], FP32, tag="ps")
                    vproj_tile(ps, slice(j * 128, (j + 1) * 128), wvh, wvl, h)
                    vs = (slice(None), j // 2, j % 2, slice(h * 512, (h + 1) * 512))
                    psum_split_f8(vh[vs], vl[vs], ps[:])
                    ci += 1

        # ---------------- Phase B1: S^T + exp -> P^T (bf16) ----------------
        # P^T[j]: [128 keys, own (8-j)*128 | other (8-j)*128] columns
        aps_cm.__exit__(None, None, None)

        # P^T pair tiles (fp8 hi/lo): pair a covers kts j=2a (sub 0) and
        # j=2a+1 (sub 1, shifted one qtile; its two leading 128-col regions
        # are zeroed so DoubleRow PV over the pair is uniformly correct).
        LN4 = 1.3862943611198906
        pth, ptl = [], []
        with tc.tile_pool(name="ptpool", bufs=1) as ptp, \
             tc.tile_pool(name="ptmp", bufs=6) as ptmpp:
            for a in range(4):
                nqa = 2 * (8 - 2 * a) * 128
                pth.append(ptp.tile([128, 2, nqa], F8, name=f"pth{a}"))
                ptl.append(ptp.tile([128, 2, nqa], F8, name=f"ptl{a}"))
            for a in range(4):
                La = (8 - 2 * a) * 128
                for t in (pth[a], ptl[a]):
                    nc.gpsimd.memset(t[:, 1, 0:128], 0.0)
                    nc.gpsimd.memset(t[:, 1, La:La + 128], 0.0)

            # B2 pools co-resident with B1's: spsum(3)+rpsum(1)+opsum(4) = 8
            with tc.tile_pool(name="opsum", bufs=cfg.get("obufs", 4), space="PSUM") as ops, \
                 tc.tile_pool(name="ostage", bufs=cfg.get("ostb", 6)) as osp, \
                 tc.tile_pool(name="rstage", bufs=1) as rsp:  # noqa: F841

                r_ps = rps.tile([128, NT], FP32)

                def emit_b1(j):
                    a, sub = j // 2, j % 2
                    La = (8 - 2 * a) * 128
                    seg = (8 - j) * 128
                    for half in range(2):      # 0 = own, 1 = other
                        q0 = j * 128 + half * 1024
                        off = half * La + sub * 128
                        done = 0
                        while done < seg:
                            cw = min(512, seg - done)
                            ps = sps.tile([128, 512], FP32, tag="s")
                            kcols = slice(j * 128, (j + 1) * 128)
                            qcols = slice(q0 + done, q0 + done + cw)
                            terms = []
                            for cc in range(4):
                                terms.append((kth[:, cc, :, kcols], qth[:, cc, :, qcols]))
                            for cc in range(4):
                                terms.append((kth[:, cc, :, kcols], qtl[:, cc, :, qcols]))
                            for cc in range(4):
                                terms.append((ktl[:, cc, :, kcols], qth[:, cc, :, qcols]))
                            for i, (lt, rt) in enumerate(terms):
                                nc.tensor.matmul(
                                    ps[:, 0:cw], lt, rt,
                                    start=(i == 0), stop=(i == len(terms) - 1),
                                    perf_mode=DR,
                                )
                            # 2-pass: exp -> fp32 tmp (Act); 0/1 mask on the
                            # SBUF tmp (DVE, off the PSUM critical path);
                            # hi on Pool, lo on DVE
                            ptmp = ptmpp.tile([128, 512], FP32, tag="pt")
                            nc.scalar.activation(
                                ptmp[:, 0:cw], ps[:, 0:cw], EXP,
                                scale=float(SCALE / (WSC * WSC)), bias=nln4_sb[:],
                            )
                            if done == 0:
                                nc.vector.tensor_tensor(
                                    ptmp[:, 0:128], ptmp[:, 0:128],
                                    mask_sb[:, half * 128:half * 128 + 128], MULT,
                                )
                            dsth = pth[a][:, sub, off + done:off + done + cw]
                            dstl = ptl[a][:, sub, off + done:off + done + cw]
                            nc.gpsimd.tensor_copy(dsth, ptmp[:, 0:cw])
                            nc.vector.tensor_tensor(dstl, ptmp[:, 0:cw], dsth, SUB)
                            done += cw

                def pv_col(ql, a):
                    La = (8 - 2 * a) * 128
                    if ql < 8:
                        return (ql - 2 * a) * 128
                    return La + (ql - 8 - 2 * a) * 128

                def emit_b2(ql, tail=False):
                    # own half: ql = j' -> jlim = j'; other: ql = 8+i -> jlim = i
                    jlim = ql if ql < 8 else ql - 8
                    alim = jlim // 2
                    o_sb = osp.tile([128, D], BF16, tag="ob")
                    for a in range(alim + 1):
                        c = pv_col(ql, a)
                        for i, pt_ in enumerate((pth[a], ptl[a])):
                            nc.tensor.matmul(
                                r_ps[:, ql:ql + 1], pt_[:, :, c:c + 128], ones_sb[:],
                                start=(a == 0 and i == 0), stop=(a == alim and i == 1),
                                perf_mode=DR, skip_group_check=True,
                            )
                    for h in range(2):
                        o_ps = ops.tile([128, 512], FP32, tag="o")
                        hs = slice(h * 512, (h + 1) * 512)
                        nterm = 3 * (alim + 1)
                        i = 0
                        for a in range(alim + 1):
                            c = pv_col(ql, a)
                            for pt_, v_ in ((pth[a], vh), (pth[a], vl), (ptl[a], vh)):
                                nc.tensor.matmul(
                                    o_ps[:],
                                    pt_[:, :, c:c + 128], v_[:, a, :, hs],
                                    start=(i == 0), stop=(i == nterm - 1),
                                    perf_mode=DR, skip_group_check=True,
                                )
                                i += 1
                        # copy each 512-half as soon as its group stops (and
                        # undo the x32 V scale); the other half's matmuls hide
                        # the copy+DMA latency
                        if tail and h == 1:
                            # final tile: quarter-split so copy/DMA pipeline
                            # and the exposed tail is one quarter, not a half
                            for q4 in range(4):
                                qs = slice(q4 * 128, (q4 + 1) * 128)
                                gs = slice(h * 512 + q4 * 128, h * 512 + (q4 + 1) * 128)
                                if q4 % 2 == 0:
                                    nc.scalar.activation(o_sb[:, gs], o_ps[:, qs], COPY, scale=INV)
                                else:
                                    nc.vector.tensor_scalar_mul(o_sb[:, gs], o_ps[:, qs], INV)
                                nc.sync.dma_start(ob_d[ql][:, gs], o_sb[:, gs])
                        elif (ql + h) % 2 == 0:
                            nc.scalar.activation(o_sb[:, hs], o_ps[:], COPY, scale=INV)
                            nc.sync.dma_start(ob_d[ql][:, hs], o_sb[:, hs])
                        else:
                            nc.vector.tensor_scalar_mul(o_sb[:, hs], o_ps[:], INV)
                            nc.sync.dma_start(ob_d[ql][:, hs], o_sb[:, hs])

                # interleave: each P^T pair a finalizes after B1 j=2a,2a+1, at
                # which point the B2 qtiles needing only pairs <= a can run.
                # Kills the B1->B2 phase boundary and spreads the copy load.
                # each wave's B2 runs one B1 iteration late so the exp->
                # hi/lo chains of its pair are fully drained when it starts
                waves = [[0, 1, 8, 9], [2, 3, 10, 11], [4, 5, 12, 13], [6, 7, 15, 14]]
                sched = [("b1", 0), ("b1", 1), ("b1", 2), ("w", 0),
                         ("b1", 3), ("b1", 4), ("w", 1),
                         ("b1", 5), ("b1", 6), ("w", 2),
                         ("b1", 7), ("w", 3)]
                for kind, idx in sched:
                    if kind == "b1":
                        emit_b1(idx)
                    else:
                        for ql in waves[idx]:
                            emit_b2(ql)
                rs_sb = rsp.tile([128, NT], FP32)
                nc.vector.tensor_copy(rs_sb[:], r_ps[:])
                nc.scalar.dma_start(rs_d, rs_sb[:])

        rps_cm.__exit__(None, None, None)
        sps_cm.__exit__(None, None, None)

    nc.compile()
    return nc


def _make_runner(nc):
    """Cached jitted 8-core runner (no donation; avoids per-call re-jit)."""
    import jax
    import numpy as np_
    from jax.sharding import Mesh, PartitionSpec
    from jax.experimental.shard_map import shard_map

    from concourse import mybir
    from concourse.bass2jax import (
        _bass_exec_p,
        install_neuronx_cc_hook,
        partition_id_tensor,
    )

    install_neuronx_cc_hook()
    partition_name = nc.partition_id_tensor.name if nc.partition_id_tensor else None
    in_names, out_names, out_avals = [], [], []
    for alloc in nc.m.functions[0].allocations:
        if not isinstance(alloc, mybir.MemoryLocationSet):
            continue
        name = alloc.memorylocations[0].name
        if alloc.kind == "ExternalInput":
            if name != partition_name:
                in_names.append(name)
        elif alloc.kind == "ExternalOutput":
            out_names.append(name)
            out_avals.append(
                jax.core.ShapedArray(
                    tuple(alloc.tensor_shape), mybir.dt.np(alloc.dtype)
                )
            )
    n_params = len(in_names)
    all_in = list(in_names) + list(out_names)
    if partition_name is not None:
        all_in.append(partition_name)

    def _body(*args):
        operands = list(args)
        if partition_name is not None:
            operands.append(partition_id_tensor())
        return tuple(
            _bass_exec_p.bind(
                *operands,
                out_avals=tuple(out_avals),
                in_names=tuple(all_in),
                out_names=tuple(out_names),
                lowering_input_output_aliases=(),
                sim_require_finite=True,
                sim_require_nnan=True,
                nc=nc,
            )
        )

    devices = jax.devices()[:NCORES]
    mesh = Mesh(np_.asarray(devices), ("core",))
    spec = PartitionSpec("core")
    fn = jax.jit(
        shard_map(
            _body,
            mesh=mesh,
            in_specs=(spec,) * (n_params + len(out_names)),
            out_specs=(spec,) * len(out_names),
            check_rep=False,
        ),
        keep_unused=True,
    )

    def run(in_maps):
        concat_in = [
            np_.concatenate([np_.asarray(m[nm]) for m in in_maps], axis=0)
            for nm in in_names
        ]
        zeros = [
            np_.zeros((NCORES * a.shape[0], *a.shape[1:]), a.dtype) for a in out_avals
        ]
        outs = fn(*concat_in, *zeros)
        return [
            {
                nm: np_.asarray(outs[i]).reshape(NCORES, *out_avals[i].shape)[c]
                for i, nm in enumerate(out_names)
            }
            for c in range(NCORES)
        ]

    return run


def _perm(p):
    return [2 * j + p for j in range(8)] + [2 * i + (1 - p) for i in range(8)]


def _split_fp8(x):
    import ml_dtypes

    f8 = ml_dtypes.float8_e4m3
    h = x.astype(f8)
    l = (x - h.astype(np.float32)).astype(f8)
    return h, l


def _pack_dlayout(t):
    """[1024 d, N] fp32 -> hi/lo fp8 [128 dpart, 4 cc, 2 di, N]."""
    n = t.shape[1]
    r = np.ascontiguousarray(
        t.reshape(4, 2, 128, n).transpose(2, 0, 1, 3)
    )
    return _split_fp8(r)


def prepare_inputs(embeddings, W_Q, W_K, W_V):
    import ml_dtypes

    emb = np.asarray(embeddings, dtype=np.float32)
    ws = {}
    for name, w in (("wq", W_Q), ("wk", W_K), ("wv", W_V)):
        wh, wl = _pack_dlayout(np.asarray(w, dtype=np.float32) * np.float32(WSC))
        ws[name + "h"], ws[name + "l"] = wh, wl

    tri = (np.arange(128)[None, :] >= np.arange(128)[:, None]).astype(np.float32)
    masks = []
    for p in range(2):
        m = np.zeros((128, 256), dtype=np.float32)
        m[:, 0:128] = tri                            # diag: 1 iff q >= k
        m[:, 128:256] = 1.0 if p == 0 else 0.0       # phantom tile mask
        masks.append(m)
    ones_np = np.ones((128, 2, 1), dtype=ml_dtypes.float8_e4m3)

    in_maps = []
    for core in range(NCORES):
        b, p = divmod(core, 2)
        x_t = emb[b].T  # [D, S]
        cols = np.concatenate([np.arange(g * 128, (g + 1) * 128) for g in _perm(p)])
        xp = np.ascontiguousarray(x_t[:, cols])
        xh, xl = _pack_dlayout(xp)
        in_maps.append(
            {
                "xh": xh, "xl": xl,
                "wqh": ws["wqh"], "wql": ws["wql"],
                "wkh": ws["wkh"], "wkl": ws["wkl"],
                "wvh": ws["wvh"], "wvl": ws["wvl"],
                "maskt": masks[p],
                "ones": ones_np,
            }
        )
    return in_maps


def merge_outputs(results):
    out = np.empty((B, S, D), dtype=np.float32)
    for b in range(B):
        osum = np.zeros((S, D), dtype=np.float32)
        rsum = np.zeros((S,), dtype=np.float32)
        for p in range(2):
            r = results[2 * b + p]
            ob = np.asarray(r["ob"]).astype(np.float32)   # [16,128,1024]
            rs = np.asarray(r["rs"]).astype(np.float32)   # [128,16]
            perm = _perm(p)
            for ql in range(NT):
                gq = perm[ql]
                osum[gq * 128:(gq + 1) * 128] += ob[ql]
                rsum[gq * 128:(gq + 1) * 128] += rs[:, ql]
        out[b] = osum / rsum[:, None]
    return out


def kernel(embeddings, W_Q, W_K, W_V):
    from concourse.bass_utils import run_bass_kernel_spmd

    if "nc" not in _CACHE:
        _CACHE["nc"] = _build()
    nc = _CACHE["nc"]

    in_maps = prepare_inputs(embeddings, W_Q, W_K, W_V)

    results = None
    try:
        if "runner" not in _CACHE:
            _CACHE["runner"] = _make_runner(nc)
        results = _CACHE["runner"](in_maps)
    except Exception:
        _CACHE.pop("runner", None)
    if results is None:
        import time as _time

        for attempt in range(2):
            try:
                results = run_bass_kernel_spmd(
                    nc, in_maps, core_ids=list(range(NCORES))
                ).results
                break
            except Exception:
                if attempt == 1:
                    raise
                _time.sleep(3.0)

    return merge_outputs(results)
